# revision 18
# baseline (speedup 1.0000x reference)
"""Trainium2 Bass kernel for nn_CrossAttention (B=8, C=512, H=W=32, Lc=1024,
8 heads x 64 dim).

Sharding: data-parallel over batch B across the 8 NeuronCores (1 image/core,
no collectives). Feature-on-partitions layout; all matmuls contract over SBUF
partitions.

v2 restructure (from the 170us v1):
  - ACT (scalar engine) runs ONLY exp during the attention phase; the
    attention steady-state is ACT-paced at ~1.15us/step x 64 steps.
  - K(m0) projection runs as soon as the (ctx, wk) tiles land; K(m1..3)
    drain into attention PE slack as fillers.
  - All stats/aux matmuls are bf16 (squares, colsums, transposes).
  - Softmax normalization: DVE reciprocal of the denominator row,
    gpsimd partition_broadcast to [64, 512], one DVE mul from PSUM ->
    bf16 ao.  No PE broadcast matmuls, no PSUM->SBUF staging copies.
  - Per-pixel RMS rows (rx for q, ry for stage D) via tiny transpose-back
    matmuls + ones-bcast matmul (rx) / partition_broadcast (ry).
  - Squares + most evicts on DVE, residual adds + broadcasts on gpsimd.
  - fp32 x copy dropped: the residual uses the bf16 x (adds ~1e-3 rel).
"""

import numpy as np
import ml_dtypes
from contextlib import ExitStack

import concourse.bass as bass
from concourse import bacc
import concourse.mybir as mybir
import concourse.tile as tile
from concourse.bass_utils import run_bass_kernel_spmd

F32 = mybir.dt.float32
F32R = mybir.dt.float32r
BF16 = mybir.dt.bfloat16
I32 = mybir.dt.int32
AF = mybir.ActivationFunctionType
OP = mybir.AluOpType

B, C, H, W = 8, 512, 32, 32
L = H * W  # 1024 query pixels
LC = 1024  # context tokens
HEADS, HD = 8, 64
VW = HD + 1  # 65: v columns + ones column (emits softmax denominator)
HID = HEADS * HD  # 512
EPS = 1e-6
NCORES = 8
CT = C // 128  # 4 c-tiles
JT = LC // 128  # 8 j-tiles


def build(dbg=False):
    nc = bacc.Bacc("TRN2", target_bir_lowering=False, debug=False,
                   num_devices=NCORES)

    x_d = nc.dram_tensor("x", [C, L], BF16, kind="ExternalInput")
    ct_d = nc.dram_tensor("ctxT", [C, LC], BF16, kind="ExternalInput")
    wq_d = nc.dram_tensor("wq", [C, HID], BF16, kind="ExternalInput")
    wk_d = nc.dram_tensor("wk", [C, HID], BF16, kind="ExternalInput")
    wv_d = nc.dram_tensor("wv", [C, HID], BF16, kind="ExternalInput")
    wo_d = nc.dram_tensor("wo", [HID, C], BF16, kind="ExternalInput")
    identb_d = nc.dram_tensor("identb", [128, 128], BF16, kind="ExternalInput")
    identr_d = nc.dram_tensor("identr", [2, 2], F32R, kind="ExternalInput")
    selxc_d = nc.dram_tensor("selxc", [128, 4], BF16, kind="ExternalInput")
    bog2_d = nc.dram_tensor("bog2T", [2, C], F32R, kind="ExternalInput")
    y_d = nc.dram_tensor("y_out", [C, L], F32, kind="ExternalOutput")
    if dbg:
        dbg_rsq = nc.dram_tensor("dbg_rsq", [128, 16], F32, kind="ExternalOutput")
        dbg_k0 = nc.dram_tensor("dbg_k0", [128, LC], F32, kind="ExternalOutput")
        dbg_q0 = nc.dram_tensor("dbg_q0", [128, L], F32, kind="ExternalOutput")
        dbg_rr = nc.dram_tensor("dbg_rr", [2, 512], F32, kind="ExternalOutput")
        dbg_bc = nc.dram_tensor("dbg_bc", [HD, 512], F32, kind="ExternalOutput")
        dbg_ao = nc.dram_tensor("dbg_ao", [128, L], F32, kind="ExternalOutput")
        dbg_ex = nc.dram_tensor("dbg_ex", [128, 1024], F32, kind="ExternalOutput")

    with tile.TileContext(nc) as tc, ExitStack() as top:
        pc = top.enter_context(tc.tile_pool(name="main", bufs=1))
        psum = top.enter_context(tc.tile_pool(name="ps", bufs=1, space="PSUM"))

        # ---------------- input DMAs ----------------
        # sync ring: x + ctx interleaved (stats need x early, K needs ctx),
        # then the last wq tiles.  scalar ring: wk + the other ctx + first
        # wq (its queue then frees for exp).  gpsimd SWDGE: wk2/3, consts,
        # wv, wo.
        x_sb = [pc.tile([128, L], BF16, tag=f"x{t}", name=f"x{t}")
                for t in range(CT)]
        ct_sb = [pc.tile([128, LC], BF16, tag=f"ct{t}", name=f"ct{t}")
                 for t in range(CT)]
        wq_sb = [pc.tile([128, HID], BF16, tag=f"wq{t}", name=f"wq{t}")
                 for t in range(CT)]
        wk_sb = [pc.tile([128, HID], BF16, tag=f"wk{t}", name=f"wk{t}")
                 for t in range(CT)]
        wv_sb = [pc.tile([128, HID], BF16, tag=f"wv{t}", name=f"wv{t}")
                 for t in range(CT)]
        wo_sb = [pc.tile([128, C], BF16, tag=f"wo{t}", name=f"wo{t}")
                 for t in range(CT)]

        warm_b = pc.tile([128, 128], BF16, tag="warmb")
        nc.vector.memset(warm_b, 1.0)
        warm_ex = pc.tile([1, 8], BF16, tag="warmex")
        nc.scalar.activation(out=warm_ex[:, :], in_=warm_b[0:1, 0:8],
                             func=AF.Exp, bias=0.0, scale=0.0)

        def dma_in(eng, sb, dram, t):
            eng.dma_start(out=sb[t], in_=dram[t * 128:(t + 1) * 128, :])

        # sync: x tiles then wq2/3; scalar: wk0, ct0, wk1, ct1, wq0/1;
        # gpsimd SWDGE: consts, wk2/ct2, wk3/ct3, wv, wo, bog2.
        dma_in(nc.sync, x_sb, x_d, 0)
        dma_in(nc.sync, x_sb, x_d, 1)
        dma_in(nc.sync, x_sb, x_d, 2)
        dma_in(nc.sync, x_sb, x_d, 3)
        dma_in(nc.sync, wq_sb, wq_d, 2)
        dma_in(nc.sync, wq_sb, wq_d, 3)
        dma_in(nc.scalar, wk_sb, wk_d, 0)
        dma_in(nc.scalar, ct_sb, ct_d, 0)
        dma_in(nc.scalar, wk_sb, wk_d, 1)
        dma_in(nc.scalar, ct_sb, ct_d, 1)
        dma_in(nc.scalar, wq_sb, wq_d, 0)
        dma_in(nc.scalar, wq_sb, wq_d, 1)
        selxc_sb = pc.tile([128, 4], BF16, tag="selxc")
        nc.gpsimd.dma_start(out=selxc_sb, in_=selxc_d[:, :])
        identb_sb = pc.tile([128, 128], BF16, tag="identb")
        nc.gpsimd.dma_start(out=identb_sb, in_=identb_d[:, :])
        dma_in(nc.gpsimd, wk_sb, wk_d, 2)
        dma_in(nc.gpsimd, ct_sb, ct_d, 2)
        dma_in(nc.gpsimd, wk_sb, wk_d, 3)
        dma_in(nc.gpsimd, ct_sb, ct_d, 3)
        for t in range(CT):
            dma_in(nc.gpsimd, wv_sb, wv_d, t)
        for t in range(CT):
            dma_in(nc.gpsimd, wo_sb, wo_d, t)
        bog2_sb = pc.tile([2, C], F32R, tag="bog2")
        nc.gpsimd.dma_start(out=bog2_sb, in_=bog2_d[:, :])
        identr_sb = pc.tile([2, 2], F32R, tag="identr")
        nc.gpsimd.dma_start(out=identr_sb, in_=identr_d[:, :])
        # ext-isa library preload for partition_broadcast (the ~6us IRAM
        # load runs invisibly before this op; keep it off the DMA path)
        warm_bc = pc.tile([2, 8], BF16, tag="warmbc")
        nc.gpsimd.partition_broadcast(warm_bc[:, :], warm_b[0:1, 0:8],
                                      channels=2)

        # ---------------- warmup + exp table load --------------------------
        # warm_b is all-ones bf16: doubles as the ones operand for the PE
        # row-broadcast matmuls.
        warm_ps = psum.tile([128, 512], F32, tag="spare", name="warmps",
                            bufs=2)
        for i in range(24):
            nc.tensor.matmul(out=warm_ps[:, 0:128],
                             lhsT=warm_b[:, :], rhs=warm_b[:, :],
                             start=True, stop=True)

        # ---------------- squares (DVE, bf16 in/out) ------------------------
        sq_x, sq_c = [], []
        for t in range(CT):
            sx = pc.tile([128, L], BF16, tag=f"sqx{t}", name=f"sqx{t}")
            sc = pc.tile([128, LC], BF16, tag=f"sqc{t}", name=f"sqc{t}")
            sq_x.append(sx)
            sq_c.append(sc)
        for xt, ct in ((0, None), (None, 0), (1, None), (2, None),
                       (None, 2), (None, 1), (3, None), (None, 3)):
            if xt is not None:
                nc.vector.tensor_mul(sq_x[xt][:, :], x_sb[xt][:, :],
                                     x_sb[xt][:, :])
            else:
                nc.vector.tensor_mul(sq_c[ct][:, :], ct_sb[ct][:, :],
                                     ct_sb[ct][:, :])

        # ---------------- K(m0) projection (early) --------------------------
        k_sb = [pc.tile([128, LC], BF16, tag=f"k{m}", name=f"k{m}")
                for m in range(CT)]
        kp0 = []
        for h in range(2):
            kp = psum.tile([128, 512], F32, tag="ou", name=f"kp0{h}", bufs=2)
            kp0.append(kp)
        for t in range(CT):
            for h in range(2):
                nc.tensor.matmul(out=kp0[h][:, :],
                                 lhsT=wk_sb[t][:, 0:128],
                                 rhs=ct_sb[t][:, h * 512:(h + 1) * 512],
                                 start=(t == 0), stop=(t == CT - 1))
        for h in range(2):
            nc.scalar.activation(out=k_sb[0][:, h * 512:(h + 1) * 512],
                                 in_=kp0[h][:, :], func=AF.Copy,
                                 bias=0.0, scale=1.0)

        # ---------------- stats colsums (bf16) ------------------------------
        # row0 = sum x^2 (per pixel), row1 = sum ctx^2 (per token); one
        # accumulation group per 512-col half, ctx parts first (they land
        # earlier than sq_x[3]).
        rows_ps = []
        for h in range(2):
            rp = psum.tile([128, 512], F32, tag="spare", name=f"rws{h}",
                           bufs=2)
            rows_ps.append(rp)
        for h in range(2):
            for t in range(CT):
                nc.tensor.matmul(out=rows_ps[h][0:2, :],
                                 lhsT=selxc_sb[:, 0:2],
                                 rhs=sq_x[t][:, h * 512:(h + 1) * 512],
                                 start=(t == 0), stop=False)
            for t in range(CT):
                nc.tensor.matmul(out=rows_ps[h][0:2, :],
                                 lhsT=selxc_sb[:, 2:4],
                                 rhs=sq_c[t][:, h * 512:(h + 1) * 512],
                                 start=False, stop=(t == CT - 1))
        rows2b = pc.tile([2, 1024], BF16, tag="rows2b")
        for h in range(2):
            nc.scalar.activation(out=rows2b[0:2, h * 512:(h + 1) * 512],
                                 in_=rows_ps[h][0:2, :], func=AF.Copy,
                                 bias=0.0, scale=1.0)

        # tiny transposes: [2, 128] chunks -> [128, 2] (ctx col, x col)
        ssq_ps = psum.tile([128, 512], F32, tag="spare", name="ssqps", bufs=2)
        for c in range(8):
            nc.tensor.matmul(out=ssq_ps[:, 2 * c:2 * c + 2],
                             lhsT=rows2b[:, c * 128:(c + 1) * 128],
                             rhs=identb_sb[0:2, 0:2],
                             start=True, stop=True)

        # Quake rsqrt on DVE: dst = (src/nfeat + eps)^-0.5, one Newton
        # pass (~0.2% max err; the downstream tolerance absorbs it).
        def dve_rsqrt(dst, src_ps, ncols, nfeat, scratch_tag):
            m = pc.tile([128, ncols], F32, tag=f"{scratch_tag}m")
            nc.vector.tensor_scalar(out=m[:, :], in0=src_ps[:, 0:ncols],
                                    scalar1=1.0 / nfeat, scalar2=EPS,
                                    op0=OP.mult, op1=OP.add)
            m2 = pc.tile([128, ncols], F32, tag=f"{scratch_tag}m2")
            nc.vector.tensor_scalar(out=m2[:, :], in0=src_ps[:, 0:ncols],
                                    scalar1=0.5 / nfeat, scalar2=0.5 * EPS,
                                    op0=OP.mult, op1=OP.add)
            i_f = pc.tile([128, ncols], F32, tag=f"{scratch_tag}if")
            nc.vector.tensor_copy(i_f[:, :], m[:, :].bitcast(I32))
            y0f = pc.tile([128, ncols], F32, tag=f"{scratch_tag}y0f")
            nc.vector.tensor_scalar(out=y0f[:, :], in0=i_f[:, :],
                                    scalar1=-0.5, scalar2=1.5974630e9,
                                    op0=OP.mult, op1=OP.add)
            y0 = pc.tile([128, ncols], I32, tag=f"{scratch_tag}y0")
            nc.vector.tensor_copy(y0[:, :], y0f[:, :])
            y0 = y0[:, :].bitcast(F32)
            t1 = pc.tile([128, ncols], F32, tag=f"{scratch_tag}t1")
            nc.vector.tensor_mul(t1[:, :], y0, y0)
            nc.vector.tensor_mul(t1[:, :], t1[:, :], m2[:, :])
            nc.vector.scalar_tensor_tensor(
                out=dst[:, :], in0=t1[:, :], scalar=1.5, in1=y0,
                op0=OP.subtract, op1=OP.mult)

        # rsq_xc cols: even = rc token chunks, odd = rx pixel chunks
        # (selxc col2 = ctx row first -> row0 = ctx, row1 = x after the
        # transpose: even col = ctx (rc), odd col = x (rx))
        rsq_xc = pc.tile([128, 16], F32, tag="rsqxc")
        dve_rsqrt(rsq_xc[:, 0:16], ssq_ps[:, 0:16], 16, C, "rs")
        rsqb = pc.tile([128, 16], BF16, tag="rsqb")
        nc.vector.tensor_copy(rsqb[:, :], rsq_xc[:, :])
        if dbg:
            nc.sync.dma_start(out=dbg_rsq[:, :], in_=rsq_xc[:, :])

        # rx broadcast: diag(rx-chunk) on DVE, then ones-bcast matmuls
        # (warm_b is all-ones); evict in two halves so q(n=0) starts early.
        bcx_ps = psum.tile([128, 1024], F32, tag="sim", name="bcxps", bufs=2)
        for c in range(8):
            dg = pc.tile([128, 128], BF16, tag="diag", name=f"dg{c}", bufs=4)
            nc.vector.tensor_scalar_mul(dg[:, :], identb_sb[:, :],
                                        rsq_xc[:, 2 * c + 1:2 * c + 2])
            nc.tensor.matmul(out=bcx_ps[:, c * 128:(c + 1) * 128],
                             lhsT=warm_b[:, :], rhs=dg[:, :],
                             start=True, stop=True)
        bcx_sb = pc.tile([128, 1024], BF16, tag="bcx")
        for h in range(2):
            nc.scalar.activation(out=bcx_sb[:, h * 512:(h + 1) * 512],
                                 in_=bcx_ps[:, h * 512:(h + 1) * 512],
                                 func=AF.Copy, bias=0.0, scale=1.0)

        # ---------------- projection machinery -----------------------------
        q_sb = [pc.tile([128, L], BF16, tag=f"q{m}", name=f"q{m}")
                for m in range(CT)]
        vT_sb = []
        for j in range(JT):
            vt = pc.tile([128, HEADS * VW], BF16, tag=f"vT{j}", name=f"vT{j}")
            vh = vt[:, :].rearrange("p (h c) -> p h c", h=HEADS)
            nc.vector.memset(vh[:, :, HD:VW], 1.0)
            vT_sb.append(vt)
        ao_sb = [pc.tile([128, L], BF16, tag=f"ao{m}", name=f"ao{m}")
                 for m in range(CT)]

        def proj_q(m, n):
            ns = slice(n * 512, (n + 1) * 512)
            ps = psum.tile([128, 512], F32, tag="spare", name=f"qp{m}{n}",
                           bufs=2)
            for t in range(CT):
                nc.tensor.matmul(out=ps[:, :],
                                 lhsT=wq_sb[t][:, m * 128:(m + 1) * 128],
                                 rhs=x_sb[t][:, ns],
                                 start=(t == 0), stop=(t == CT - 1))
            nc.vector.tensor_mul(q_sb[m][:, ns], ps[:, :], bcx_sb[:, ns])

        def proj_v(j):
            ps = psum.tile([128, HID], F32, tag="spare", name=f"vp{j}",
                           bufs=2)
            for t in range(CT):
                nc.tensor.matmul(out=ps[:, :],
                                 lhsT=ct_sb[t][:, j * 128:(j + 1) * 128],
                                 rhs=wv_sb[t][:, :],
                                 start=(t == 0), stop=(t == CT - 1))
            vh = vT_sb[j][:, :].rearrange("p (h c) -> p h c", h=HEADS)
            nc.vector.tensor_scalar_mul(
                vh[:, :, 0:HD],
                ps[:, :].rearrange("p (h c) -> p h c", h=HEADS),
                rsq_xc[:, 2 * j:2 * j + 1])

        def proj_k_half(m, h):
            ps = psum.tile([128, 512], F32, tag="spare", name=f"kp{m}{h}",
                           bufs=2)
            for t in range(CT):
                nc.tensor.matmul(out=ps[:, :],
                                 lhsT=wk_sb[t][:, m * 128:(m + 1) * 128],
                                 rhs=ct_sb[t][:, h * 512:(h + 1) * 512],
                                 start=(t == 0), stop=(t == CT - 1))
            nc.vector.tensor_copy(k_sb[m][:, h * 512:(h + 1) * 512],
                                  ps[:, :])

        def bog_transposes():
            bog_res = []
            for t in range(CT):
                bps = psum.tile([128, 512], F32, tag="spare", name=f"bog{t}",
                                bufs=2)
                nc.tensor.matmul(out=bps[:, 0:2],
                                 lhsT=bog2_sb[:, t * 128:(t + 1) * 128],
                                 rhs=identr_sb[0:2, 0:2],
                                 start=True, stop=True)
                bg = pc.tile([128, 2], F32, tag=f"bog2s{t}")
                nc.vector.tensor_copy(bg[:, :], bps[:, 0:2])
                bog_res.append(bg)
            for bg in bog_res:
                bo_sb.append(bg[:, 0:1])
                g2_sb.append(bg[:, 1:2])
        bo_sb, g2_sb = [], []

        # pre-attention projections (q n=0 and the first v tiles)
        proj_q(0, 0)
        proj_q(1, 0)
        proj_v(0)
        proj_v(1)
        proj_q(2, 0)
        proj_q(3, 0)

        # deferred work, drained into attention PE slack.  Order matters:
        # vT[j] must be emitted before the PV that reads it (PV(0,0,j) at
        # step j), k_sb[p] before pair p's sims (emitted one step early).
        filler = [
            lambda: proj_v(2),
            lambda: proj_v(3),
            lambda: proj_v(4),
            lambda: proj_v(5),
            lambda: proj_v(6),
            lambda: proj_v(7),
            lambda: proj_k_half(1, 0),
            lambda: proj_k_half(1, 1),
            lambda: proj_k_half(2, 0),
            bog_transposes,
            lambda: proj_k_half(2, 1),
            lambda: proj_k_half(3, 0),
            lambda: proj_k_half(3, 1),
            lambda: proj_q(0, 1),
            lambda: proj_q(1, 1),
            lambda: proj_q(2, 1),
            lambda: proj_q(3, 1),
        ]

        # ---------------- stage D (emitted later, per n) --------------------
        ybig = pc.tile([128, 4 * L], F32, tag="ybig")
        ysq_t = [pc.tile([128, 512], BF16, tag=f"ysq{m}", name=f"ysq{m}")
                 for m in range(CT)]
        bcy_cur = {}

        def stage_d(n):
            ns = slice(n * 512, (n + 1) * 512)
            ops = []
            for m in range(CT):
                def dproj(m=m):
                    ps = psum.tile([128, 512], F32, tag="spare",
                                   name=f"yp{m}{n}", bufs=2)
                    for t in range(CT):
                        nc.tensor.matmul(
                            out=ps[:, :],
                            lhsT=wo_sb[t][:, m * 128:(m + 1) * 128],
                            rhs=ao_sb[t][:, ns],
                            start=(t == 0), stop=(t == CT - 1))
                    ysl = ybig[:, m * L + n * 512: m * L + (n + 1) * 512]
                    nc.vector.tensor_scalar_add(ysl, ps[:, :], bo_sb[m])
                    nc.vector.tensor_mul(ysq_t[m][:, :], ysl, ysl)
                ops.append(dproj)

            dst_state = {}

            def dstat1():
                yr = psum.tile([128, 512], F32, tag="spare", bufs=2,
                               name=f"yr{n}")
                for m in range(CT):
                    nc.tensor.matmul(out=yr[0:1, :],
                                     lhsT=selxc_sb[:, 1:2],
                                     rhs=ysq_t[m][:, :],
                                     start=(m == 0), stop=(m == CT - 1))
                rowyb = pc.tile([1, 512], BF16, tag="rowyb", name=f"rwy{n}",
                                bufs=2)
                nc.vector.tensor_copy(rowyb[0:1, :], yr[0:1, :])
                syp = psum.tile([128, 512], F32, tag="spare", bufs=2,
                                name=f"syp{n}")
                for c in range(4):
                    nc.tensor.matmul(out=syp[:, c:c + 1],
                                     lhsT=rowyb[:, c * 128:(c + 1) * 128],
                                     rhs=identb_sb[0:1, 0:1],
                                     start=True, stop=True)
                dst_state["syp"] = syp
            ops.append(dstat1)

            def dstat2():
                syp = dst_state.pop("syp")
                ry = pc.tile([128, 4], F32, tag="ryq", name=f"ry{n}", bufs=2)
                dve_rsqrt(ry, syp, 4, C, f"ry{n}")
                ryb = pc.tile([128, 4], BF16, tag="rybq", name=f"ryb{n}",
                              bufs=2)
                nc.vector.tensor_copy(ryb[:, :], ry[:, :])
                ryrow_ps = psum.tile([128, 512], F32, tag="spare",
                                     name=f"ryr{n}", bufs=2)
                for c in range(4):
                    nc.tensor.matmul(out=ryrow_ps[0:1, c * 128:(c + 1) * 128],
                                     lhsT=ryb[:, c:c + 1],
                                     rhs=identb_sb[:, :],
                                     start=True, stop=True)
                ryrowb = pc.tile([1, 512], BF16, tag="ryrowb",
                                 name=f"ryrb{n}", bufs=2)
                nc.vector.tensor_copy(ryrowb[:, :], ryrow_ps[0:1, :])
                bcy = pc.tile([128, 512], BF16, tag="bcy", name=f"bcy{n}",
                              bufs=2)
                nc.gpsimd.partition_broadcast(bcy[:, :], ryrowb[0:1, :],
                                              channels=128)
                bcy_cur[0] = bcy
            ops.append(dstat2)

            for m in range(CT):
                def dfin(m=m):
                    ysl = ybig[:, m * L + n * 512: m * L + (n + 1) * 512]
                    tmp = pc.tile([128, 512], F32, tag="fintmp",
                                  name=f"ft{n}{m}", bufs=2)
                    nc.vector.scalar_tensor_tensor(
                        out=tmp[:, :], in0=ysl, scalar=g2_sb[m],
                        in1=bcy_cur[0][:, :], op0=OP.mult, op1=OP.mult)
                    nc.vector.tensor_add(ysl, tmp[:, :], x_sb[m][:, ns])
                    nc.sync.dma_start(
                        out=y_d[m * 128:(m + 1) * 128, ns], in_=ysl)
                ops.append(dfin)
            return ops

        # ---------------- attention ----------------------------------------
        pexp = top.enter_context(tc.tile_pool(name="exp", bufs=1))

        steps = [(n, p, j) for n in range(2) for p in range(4)
                 for j in range(JT)]

        sim_slots = {}
        ex_slots = {}

        def emit_sims(step):
            n, p, j = step
            ns = slice(n * 512, (n + 1) * 512)
            js = slice(j * 128, (j + 1) * 128)
            sl = psum.tile([128, 1024], F32, tag="sim", bufs=2,
                           name=f"sim{n}{p}{j}")
            nc.tensor.matmul(out=sl[:, 0:512],
                             lhsT=k_sb[p][0:64, js],
                             rhs=q_sb[p][0:64, ns],
                             start=True, stop=True)
            nc.tensor.matmul(out=sl[:, 512:1024],
                             lhsT=k_sb[p][64:128, js],
                             rhs=q_sb[p][64:128, ns],
                             start=True, stop=True)
            sim_slots[step] = sl

        def emit_exps(step):
            n, p, j = step
            ex = pexp.tile([128, 1024], BF16, tag="ex", bufs=4,
                           name=f"ex{n}{p}{j}")
            nc.scalar.activation(out=ex[:, :], in_=sim_slots.pop(step)[:, :],
                                 func=AF.Exp, bias=0.0,
                                 scale=rsq_xc[:, 2 * j:2 * j + 1])
            ex_slots[step] = ex
            if dbg and step == (0, 0, 0):
                exf = pc.tile([128, 1024], F32, tag="dbgexf")
                nc.vector.tensor_copy(exf[:, :], ex[:, :])
                nc.sync.dma_start(out=dbg_ex[:, :], in_=exf[:, :])

        ou_cur = {}

        def emit_pv(step):
            n, p, j = step
            if j == 0:
                ou_cur[0] = psum.tile([128, 512], F32, tag="ou", bufs=2,
                                      name=f"ou{n}{p}0")
                ou_cur[1] = psum.tile([128, 512], F32, tag="ou", bufs=2,
                                      name=f"ou{n}{p}1")
            ex = ex_slots.pop(step)
            for hi in range(2):
                h = 2 * p + hi  # global head
                nc.tensor.matmul(
                    out=ou_cur[hi][0:VW, :],
                    lhsT=vT_sb[j][:, h * VW:(h + 1) * VW],
                    rhs=ex[:, hi * 512:(hi + 1) * 512],
                    start=(j == 0), stop=(j == JT - 1))

        def emit_pair_norm1(step):
            # reciprocal of the denominator rows + partition broadcast.
            # gpsimd runs ONLY partition_broadcast (ext-isa lib) -- mixing
            # it with stock tensor ops thrashes the Q7 IRAM library.
            n, p, j = step
            ous, bcs = [], []
            for hi in range(2):
                ou = ou_cur[hi]
                rden = pc.tile([1, 512], F32, tag="rden",
                               name=f"rd{n}{p}{hi}", bufs=4)
                nc.vector.tensor_copy(rden[:, :], ou[HD:VW, :])
                rr = pc.tile([1, 512], F32, tag="rr", name=f"rr{n}{p}{hi}",
                             bufs=4)
                nc.vector.reciprocal_approx_fast(out=rr[:, :],
                                                 in_=rden[:, :])
                bc = pc.tile([HD, 512], F32, tag="bcd", name=f"bc{n}{p}{hi}",
                             bufs=4)
                nc.gpsimd.partition_broadcast(bc[:, :], rr[0:1, :],
                                              channels=HD)
                ous.append(ou)
                bcs.append(bc)
            return (ous, bcs)

        def emit_pair_norm2(step, ous, bcs):
            n, p, j = step
            ns = slice(n * 512, (n + 1) * 512)
            for hi in range(2):
                nc.vector.tensor_mul(
                    ao_sb[p][hi * HD:(hi + 1) * HD, ns],
                    ous[hi][0:HD, :], bcs[hi][:, :])

        # ---- emission with software pipelining ----
        d_ops = []
        pend2 = None
        emit_sims(steps[0])
        for si, step in enumerate(steps):
            n, p, j = step
            if si >= 36 and d_ops:
                d_ops.pop(0)()
            elif si >= 1 and filler:
                filler.pop(0)()
            emit_exps(step)
            if si + 1 < len(steps):
                emit_sims(steps[si + 1])
            if pend2 is not None:
                emit_pair_norm2(*pend2)
                pend2 = None
            emit_pv(step)
            if j == JT - 1:
                pend2 = (step,) + emit_pair_norm1(step)
                if (n, p) == (0, 3):
                    d_ops = stage_d(0)
        if pend2 is not None:
            emit_pair_norm2(*pend2)
        for op in d_ops:
            op()
        for op in stage_d(1):
            op()
        if dbg:
            aof = pc.tile([128, L], F32, tag="dbgaof")
            nc.vector.tensor_copy(aof[:, :], ao_sb[0][:, :])
            nc.sync.dma_start(out=dbg_ao[:, :], in_=aof[:, :])

    nc.compile()
    return nc


_NC_CACHE = {}


def _get_nc():
    if "nc" not in _NC_CACHE:
        _NC_CACHE["nc"] = build()
    return _NC_CACHE["nc"]


def kernel(x, context, Wq, Wkv, Wo, bo, g, g2):
    x = np.asarray(x, dtype=np.float32)
    context = np.asarray(context, dtype=np.float32)
    Wq = np.asarray(Wq, dtype=np.float32)
    Wkv = np.asarray(Wkv, dtype=np.float32)
    Wo = np.asarray(Wo, dtype=np.float32)
    bo = np.asarray(bo, dtype=np.float32)
    g = np.asarray(g, dtype=np.float32)
    g2 = np.asarray(g2, dtype=np.float32)

    bf = ml_dtypes.bfloat16
    scale = HD ** -0.5
    wq_h = np.ascontiguousarray((Wq * g[None, :] * scale).T).astype(bf)
    wk_h = np.ascontiguousarray((Wkv[:HID] * g[None, :]).T).astype(bf)
    wv_h = np.ascontiguousarray((Wkv[HID:] * g[None, :]).T).astype(bf)
    wo_h = np.ascontiguousarray(Wo.T).astype(bf)
    bog2T = np.ascontiguousarray(np.stack([bo, g2], axis=0))  # [2, C]
    identb = np.eye(128, dtype=np.float32).astype(bf)
    identr = np.eye(2, dtype=np.float32)
    # stats rows: ctx -> row 0 (rc at even cols after transpose),
    # x -> row 1 (rx at odd cols)
    selxc = np.zeros((128, 4), dtype=np.float32)
    selxc[:, 1] = 1.0   # x part (lhsT [:, 0:2]): col1 -> row 1
    selxc[:, 2] = 1.0   # ctx part (lhsT [:, 2:4]): col2 -> row 0
    selxc = selxc.astype(bf)

    nc = _get_nc()
    global _last_in_maps
    in_maps = []
    for i in range(NCORES):
        in_maps.append({
            "x": np.ascontiguousarray(x[i].reshape(C, L)).astype(bf),
            "ctxT": np.ascontiguousarray(context[i].T).astype(bf),
            "wq": wq_h, "wk": wk_h, "wv": wv_h, "wo": wo_h,
            "identb": identb, "identr": identr, "selxc": selxc,
            "bog2T": bog2T,
        })
    _last_in_maps = in_maps
    res = run_bass_kernel_spmd(nc, in_maps, list(range(NCORES)))
    out = np.stack([res.results[i]["y_out"].reshape(C, H, W)
                    for i in range(NCORES)])
    return out.astype(np.float32)


_last_in_maps = None


# revision 20
# speedup vs baseline: 1.0133x; 1.0133x over previous
"""Trainium2 Bass kernel for nn_CrossAttention (B=8, C=512, H=W=32, Lc=1024,
8 heads x 64 dim).

Sharding: data-parallel over batch B across the 8 NeuronCores (1 image/core,
no collectives). Feature-on-partitions layout; all matmuls contract over SBUF
partitions.

v2 restructure (from the 170us v1):
  - ACT (scalar engine) runs ONLY exp during the attention phase; the
    attention steady-state is ACT-paced at ~1.15us/step x 64 steps.
  - K(m0) projection runs as soon as the (ctx, wk) tiles land; K(m1..3)
    drain into attention PE slack as fillers.
  - All stats/aux matmuls are bf16 (squares, colsums, transposes).
  - Softmax normalization: DVE reciprocal of the denominator row,
    gpsimd partition_broadcast to [64, 512], one DVE mul from PSUM ->
    bf16 ao.  No PE broadcast matmuls, no PSUM->SBUF staging copies.
  - Per-pixel RMS rows (rx for q, ry for stage D) via tiny transpose-back
    matmuls + ones-bcast matmul (rx) / partition_broadcast (ry).
  - Squares + most evicts on DVE, residual adds + broadcasts on gpsimd.
  - fp32 x copy dropped: the residual uses the bf16 x (adds ~1e-3 rel).
"""

import numpy as np
import ml_dtypes
from contextlib import ExitStack

import concourse.bass as bass
from concourse import bacc
import concourse.mybir as mybir
import concourse.tile as tile
from concourse.bass_utils import run_bass_kernel_spmd

F32 = mybir.dt.float32
F32R = mybir.dt.float32r
BF16 = mybir.dt.bfloat16
I32 = mybir.dt.int32
AF = mybir.ActivationFunctionType
OP = mybir.AluOpType

B, C, H, W = 8, 512, 32, 32
L = H * W  # 1024 query pixels
LC = 1024  # context tokens
HEADS, HD = 8, 64
VW = HD + 1  # 65: v columns + ones column (emits softmax denominator)
HID = HEADS * HD  # 512
EPS = 1e-6
NCORES = 8
CT = C // 128  # 4 c-tiles
JT = LC // 128  # 8 j-tiles


def build(dbg=False):
    nc = bacc.Bacc("TRN2", target_bir_lowering=False, debug=False,
                   num_devices=NCORES)

    x_d = nc.dram_tensor("x", [C, L], BF16, kind="ExternalInput")
    ct_d = nc.dram_tensor("ctxT", [C, LC], BF16, kind="ExternalInput")
    wq_d = nc.dram_tensor("wq", [C, HID], BF16, kind="ExternalInput")
    wk_d = nc.dram_tensor("wk", [C, HID], BF16, kind="ExternalInput")
    wv_d = nc.dram_tensor("wv", [C, HID], BF16, kind="ExternalInput")
    wo_d = nc.dram_tensor("wo", [HID, C], BF16, kind="ExternalInput")
    identb_d = nc.dram_tensor("identb", [128, 128], BF16, kind="ExternalInput")
    identr_d = nc.dram_tensor("identr", [2, 2], F32R, kind="ExternalInput")
    selxc_d = nc.dram_tensor("selxc", [128, 4], BF16, kind="ExternalInput")
    bog2_d = nc.dram_tensor("bog2T", [2, C], F32R, kind="ExternalInput")
    y_d = nc.dram_tensor("y_out", [C, L], F32, kind="ExternalOutput")
    if dbg:
        dbg_rsq = nc.dram_tensor("dbg_rsq", [128, 16], F32, kind="ExternalOutput")
        dbg_k0 = nc.dram_tensor("dbg_k0", [128, LC], F32, kind="ExternalOutput")
        dbg_q0 = nc.dram_tensor("dbg_q0", [128, L], F32, kind="ExternalOutput")
        dbg_rr = nc.dram_tensor("dbg_rr", [2, 512], F32, kind="ExternalOutput")
        dbg_bc = nc.dram_tensor("dbg_bc", [HD, 512], F32, kind="ExternalOutput")
        dbg_ao = nc.dram_tensor("dbg_ao", [128, L], F32, kind="ExternalOutput")
        dbg_ex = nc.dram_tensor("dbg_ex", [128, 1024], F32, kind="ExternalOutput")

    with tile.TileContext(nc) as tc, ExitStack() as top:
        pc = top.enter_context(tc.tile_pool(name="main", bufs=1))
        psum = top.enter_context(tc.tile_pool(name="ps", bufs=1, space="PSUM"))

        # ---------------- input DMAs ----------------
        # sync ring: x + ctx interleaved (stats need x early, K needs ctx),
        # then the last wq tiles.  scalar ring: wk + the other ctx + first
        # wq (its queue then frees for exp).  gpsimd SWDGE: wk2/3, consts,
        # wv, wo.
        x_sb = [pc.tile([128, L], BF16, tag=f"x{t}", name=f"x{t}")
                for t in range(CT)]
        ct_sb = [pc.tile([128, LC], BF16, tag=f"ct{t}", name=f"ct{t}")
                 for t in range(CT)]
        wq_sb = [pc.tile([128, HID], BF16, tag=f"wq{t}", name=f"wq{t}")
                 for t in range(CT)]
        wk_sb = [pc.tile([128, HID], BF16, tag=f"wk{t}", name=f"wk{t}")
                 for t in range(CT)]
        wv_sb = [pc.tile([128, HID], BF16, tag=f"wv{t}", name=f"wv{t}")
                 for t in range(CT)]
        wo_sb = [pc.tile([128, C], BF16, tag=f"wo{t}", name=f"wo{t}")
                 for t in range(CT)]

        warm_b = pc.tile([128, 128], BF16, tag="warmb")
        nc.vector.memset(warm_b, 1.0)
        warm_ex = pc.tile([1, 8], BF16, tag="warmex")
        nc.scalar.activation(out=warm_ex[:, :], in_=warm_b[0:1, 0:8],
                             func=AF.Exp, bias=0.0, scale=0.0)

        def dma_in(eng, sb, dram, t):
            eng.dma_start(out=sb[t], in_=dram[t * 128:(t + 1) * 128, :])

        # sync: x tiles then wq2/3; scalar: wk0, ct0, wk1, ct1, wq0/1;
        # gpsimd SWDGE: consts, wk2/ct2, wk3/ct3, wv, wo, bog2.
        dma_in(nc.sync, x_sb, x_d, 0)
        dma_in(nc.sync, x_sb, x_d, 1)
        dma_in(nc.sync, x_sb, x_d, 2)
        dma_in(nc.sync, x_sb, x_d, 3)
        dma_in(nc.sync, wq_sb, wq_d, 2)
        dma_in(nc.sync, wq_sb, wq_d, 3)
        dma_in(nc.scalar, wk_sb, wk_d, 0)
        dma_in(nc.scalar, ct_sb, ct_d, 0)
        dma_in(nc.scalar, wk_sb, wk_d, 1)
        dma_in(nc.scalar, ct_sb, ct_d, 1)
        dma_in(nc.scalar, wq_sb, wq_d, 0)
        dma_in(nc.scalar, wq_sb, wq_d, 1)
        selxc_sb = pc.tile([128, 4], BF16, tag="selxc")
        nc.gpsimd.dma_start(out=selxc_sb, in_=selxc_d[:, :])
        identb_sb = pc.tile([128, 128], BF16, tag="identb")
        nc.gpsimd.dma_start(out=identb_sb, in_=identb_d[:, :])
        dma_in(nc.gpsimd, wk_sb, wk_d, 2)
        dma_in(nc.gpsimd, ct_sb, ct_d, 2)
        dma_in(nc.gpsimd, wk_sb, wk_d, 3)
        dma_in(nc.gpsimd, ct_sb, ct_d, 3)
        for t in range(CT):
            dma_in(nc.gpsimd, wv_sb, wv_d, t)
        for t in range(CT):
            dma_in(nc.gpsimd, wo_sb, wo_d, t)
        bog2_sb = pc.tile([2, C], F32R, tag="bog2")
        nc.gpsimd.dma_start(out=bog2_sb, in_=bog2_d[:, :])
        identr_sb = pc.tile([2, 2], F32R, tag="identr")
        nc.gpsimd.dma_start(out=identr_sb, in_=identr_d[:, :])
        # ext-isa library preload for partition_broadcast (the ~6us IRAM
        # load runs invisibly before this op; keep it off the DMA path)
        warm_bc = pc.tile([2, 8], BF16, tag="warmbc")
        nc.gpsimd.partition_broadcast(warm_bc[:, :], warm_b[0:1, 0:8],
                                      channels=2)

        # ---------------- warmup + exp table load --------------------------
        # warm_b is all-ones bf16: doubles as the ones operand for the PE
        # row-broadcast matmuls.
        warm_ps = psum.tile([128, 512], F32, tag="spare", name="warmps",
                            bufs=2)
        for i in range(24):
            nc.tensor.matmul(out=warm_ps[:, 0:128],
                             lhsT=warm_b[:, :], rhs=warm_b[:, :],
                             start=True, stop=True)

        # ---------------- squares (DVE, bf16 in/out) ------------------------
        sq_x, sq_c = [], []
        for t in range(CT):
            sx = pc.tile([128, L], BF16, tag=f"sqx{t}", name=f"sqx{t}")
            sc = pc.tile([128, LC], BF16, tag=f"sqc{t}", name=f"sqc{t}")
            sq_x.append(sx)
            sq_c.append(sc)
        for xt, ct in ((0, None), (None, 0), (1, None), (2, None),
                       (None, 2), (None, 1), (3, None), (None, 3)):
            if xt is not None:
                nc.vector.tensor_mul(sq_x[xt][:, :], x_sb[xt][:, :],
                                     x_sb[xt][:, :])
            else:
                nc.vector.tensor_mul(sq_c[ct][:, :], ct_sb[ct][:, :],
                                     ct_sb[ct][:, :])

        # ---------------- K(m0) projection (early) --------------------------
        k_sb = [pc.tile([128, LC], BF16, tag=f"k{m}", name=f"k{m}")
                for m in range(CT)]
        kp0 = []
        for h in range(2):
            kp = psum.tile([128, 512], F32, tag="ou", name=f"kp0{h}", bufs=2)
            kp0.append(kp)
        for ti, t in enumerate((0, 2, 1, 3)):
            for h in range(2):
                nc.tensor.matmul(out=kp0[h][:, :],
                                 lhsT=wk_sb[t][:, 0:128],
                                 rhs=ct_sb[t][:, h * 512:(h + 1) * 512],
                                 start=(ti == 0), stop=(ti == CT - 1))
        for h in range(2):
            nc.scalar.activation(out=k_sb[0][:, h * 512:(h + 1) * 512],
                                 in_=kp0[h][:, :], func=AF.Copy,
                                 bias=0.0, scale=1.0)

        # ---------------- stats colsums (bf16) ------------------------------
        # row0 = sum x^2 (per pixel), row1 = sum ctx^2 (per token); one
        # accumulation group per 512-col half, ctx parts first (they land
        # earlier than sq_x[3]).
        rows_ps = []
        for h in range(2):
            rp = psum.tile([128, 512], F32, tag="spare", name=f"rws{h}",
                           bufs=2)
            rows_ps.append(rp)
        for h in range(2):
            for t in range(CT):
                nc.tensor.matmul(out=rows_ps[h][0:2, :],
                                 lhsT=selxc_sb[:, 0:2],
                                 rhs=sq_x[t][:, h * 512:(h + 1) * 512],
                                 start=(t == 0), stop=False)
            for t in range(CT):
                nc.tensor.matmul(out=rows_ps[h][0:2, :],
                                 lhsT=selxc_sb[:, 2:4],
                                 rhs=sq_c[t][:, h * 512:(h + 1) * 512],
                                 start=False, stop=(t == CT - 1))
        rows2b = pc.tile([2, 1024], BF16, tag="rows2b")
        for h in range(2):
            nc.scalar.activation(out=rows2b[0:2, h * 512:(h + 1) * 512],
                                 in_=rows_ps[h][0:2, :], func=AF.Copy,
                                 bias=0.0, scale=1.0)

        # tiny transposes: [2, 128] chunks -> [128, 2] (ctx col, x col)
        ssq_ps = psum.tile([128, 512], F32, tag="spare", name="ssqps", bufs=2)
        for c in range(8):
            nc.tensor.matmul(out=ssq_ps[:, 2 * c:2 * c + 2],
                             lhsT=rows2b[:, c * 128:(c + 1) * 128],
                             rhs=identb_sb[0:2, 0:2],
                             start=True, stop=True)

        # Quake rsqrt on DVE: dst = (src/nfeat + eps)^-0.5, one Newton
        # pass (~0.2% max err; the downstream tolerance absorbs it).
        def dve_rsqrt(dst, src_ps, ncols, nfeat, scratch_tag):
            m = pc.tile([128, ncols], F32, tag=f"{scratch_tag}m")
            nc.vector.tensor_scalar(out=m[:, :], in0=src_ps[:, 0:ncols],
                                    scalar1=1.0 / nfeat, scalar2=EPS,
                                    op0=OP.mult, op1=OP.add)
            m2 = pc.tile([128, ncols], F32, tag=f"{scratch_tag}m2")
            nc.vector.tensor_scalar(out=m2[:, :], in0=src_ps[:, 0:ncols],
                                    scalar1=0.5 / nfeat, scalar2=0.5 * EPS,
                                    op0=OP.mult, op1=OP.add)
            i_f = pc.tile([128, ncols], F32, tag=f"{scratch_tag}if")
            nc.vector.tensor_copy(i_f[:, :], m[:, :].bitcast(I32))
            y0f = pc.tile([128, ncols], F32, tag=f"{scratch_tag}y0f")
            nc.vector.tensor_scalar(out=y0f[:, :], in0=i_f[:, :],
                                    scalar1=-0.5, scalar2=1.5974630e9,
                                    op0=OP.mult, op1=OP.add)
            y0 = pc.tile([128, ncols], I32, tag=f"{scratch_tag}y0")
            nc.vector.tensor_copy(y0[:, :], y0f[:, :])
            y0 = y0[:, :].bitcast(F32)
            t1 = pc.tile([128, ncols], F32, tag=f"{scratch_tag}t1")
            nc.vector.tensor_mul(t1[:, :], y0, y0)
            nc.vector.tensor_mul(t1[:, :], t1[:, :], m2[:, :])
            nc.vector.scalar_tensor_tensor(
                out=dst[:, :], in0=t1[:, :], scalar=1.5, in1=y0,
                op0=OP.subtract, op1=OP.mult)

        # rsq_xc cols: even = rc token chunks, odd = rx pixel chunks
        # (selxc col2 = ctx row first -> row0 = ctx, row1 = x after the
        # transpose: even col = ctx (rc), odd col = x (rx))
        rsq_xc = pc.tile([128, 16], F32, tag="rsqxc")
        dve_rsqrt(rsq_xc[:, 0:16], ssq_ps[:, 0:16], 16, C, "rs")
        rsqb = pc.tile([128, 16], BF16, tag="rsqb")
        nc.vector.tensor_copy(rsqb[:, :], rsq_xc[:, :])
        if dbg:
            nc.sync.dma_start(out=dbg_rsq[:, :], in_=rsq_xc[:, :])

        # rx broadcast: diag(rx-chunk) on DVE, then ones-bcast matmuls
        # (warm_b is all-ones); evict in two halves so q(n=0) starts early.
        bcx_ps = psum.tile([128, 1024], F32, tag="sim", name="bcxps", bufs=2)
        for c in range(8):
            dg = pc.tile([128, 128], BF16, tag="diag", name=f"dg{c}", bufs=4)
            nc.vector.tensor_scalar_mul(dg[:, :], identb_sb[:, :],
                                        rsq_xc[:, 2 * c + 1:2 * c + 2])
            nc.tensor.matmul(out=bcx_ps[:, c * 128:(c + 1) * 128],
                             lhsT=warm_b[:, :], rhs=dg[:, :],
                             start=True, stop=True)
        bcx_sb = pc.tile([128, 1024], BF16, tag="bcx")
        for h in range(2):
            nc.scalar.activation(out=bcx_sb[:, h * 512:(h + 1) * 512],
                                 in_=bcx_ps[:, h * 512:(h + 1) * 512],
                                 func=AF.Copy, bias=0.0, scale=1.0)

        # ---------------- projection machinery -----------------------------
        q_sb = [pc.tile([128, L], BF16, tag=f"q{m}", name=f"q{m}")
                for m in range(CT)]
        vT_sb = []
        for j in range(JT):
            vt = pc.tile([128, HEADS * VW], BF16, tag=f"vT{j}", name=f"vT{j}")
            vh = vt[:, :].rearrange("p (h c) -> p h c", h=HEADS)
            nc.vector.memset(vh[:, :, HD:VW], 1.0)
            vT_sb.append(vt)
        ao_sb = [pc.tile([128, L], BF16, tag=f"ao{m}", name=f"ao{m}")
                 for m in range(CT)]

        def proj_q(m, n):
            ns = slice(n * 512, (n + 1) * 512)
            ps = psum.tile([128, 512], F32, tag="spare", name=f"qp{m}{n}",
                           bufs=2)
            for t in range(CT):
                nc.tensor.matmul(out=ps[:, :],
                                 lhsT=wq_sb[t][:, m * 128:(m + 1) * 128],
                                 rhs=x_sb[t][:, ns],
                                 start=(t == 0), stop=(t == CT - 1))
            nc.vector.tensor_mul(q_sb[m][:, ns], ps[:, :], bcx_sb[:, ns])

        def proj_v(j):
            ps = psum.tile([128, HID], F32, tag="spare", name=f"vp{j}",
                           bufs=2)
            for t in range(CT):
                nc.tensor.matmul(out=ps[:, :],
                                 lhsT=ct_sb[t][:, j * 128:(j + 1) * 128],
                                 rhs=wv_sb[t][:, :],
                                 start=(t == 0), stop=(t == CT - 1))
            vh = vT_sb[j][:, :].rearrange("p (h c) -> p h c", h=HEADS)
            nc.vector.tensor_scalar_mul(
                vh[:, :, 0:HD],
                ps[:, :].rearrange("p (h c) -> p h c", h=HEADS),
                rsq_xc[:, 2 * j:2 * j + 1])

        def proj_k_half(m, h):
            ps = psum.tile([128, 512], F32, tag="spare", name=f"kp{m}{h}",
                           bufs=2)
            for t in range(CT):
                nc.tensor.matmul(out=ps[:, :],
                                 lhsT=wk_sb[t][:, m * 128:(m + 1) * 128],
                                 rhs=ct_sb[t][:, h * 512:(h + 1) * 512],
                                 start=(t == 0), stop=(t == CT - 1))
            nc.vector.tensor_copy(k_sb[m][:, h * 512:(h + 1) * 512],
                                  ps[:, :])

        def bog_transposes():
            bog_res = []
            for t in range(CT):
                bps = psum.tile([128, 512], F32, tag="spare", name=f"bog{t}",
                                bufs=2)
                nc.tensor.matmul(out=bps[:, 0:2],
                                 lhsT=bog2_sb[:, t * 128:(t + 1) * 128],
                                 rhs=identr_sb[0:2, 0:2],
                                 start=True, stop=True)
                bg = pc.tile([128, 2], F32, tag=f"bog2s{t}")
                nc.vector.tensor_copy(bg[:, :], bps[:, 0:2])
                bog_res.append(bg)
            for bg in bog_res:
                bo_sb.append(bg[:, 0:1])
                g2_sb.append(bg[:, 1:2])
        bo_sb, g2_sb = [], []

        # pre-attention projections (q n=0 and the first v tiles)
        proj_q(0, 0)
        proj_q(1, 0)
        proj_v(0)
        proj_v(1)
        proj_q(2, 0)
        proj_q(3, 0)

        # deferred work, drained into attention PE slack.  Order matters:
        # vT[j] must be emitted before the PV that reads it (PV(0,0,j) at
        # step j), k_sb[p] before pair p's sims (emitted one step early).
        filler = [
            lambda: proj_v(2),
            lambda: proj_v(3),
            lambda: proj_v(4),
            lambda: proj_v(5),
            lambda: proj_v(6),
            lambda: proj_v(7),
            lambda: proj_k_half(1, 0),
            lambda: proj_k_half(1, 1),
            lambda: proj_k_half(2, 0),
            bog_transposes,
            lambda: proj_k_half(2, 1),
            lambda: proj_k_half(3, 0),
            lambda: proj_k_half(3, 1),
            lambda: proj_q(0, 1),
            lambda: proj_q(1, 1),
            lambda: proj_q(2, 1),
            lambda: proj_q(3, 1),
        ]

        # ---------------- stage D (emitted later, per n) --------------------
        ybig = pc.tile([128, 4 * L], F32, tag="ybig")
        ysq_t = [pc.tile([128, 512], BF16, tag=f"ysq{m}", name=f"ysq{m}")
                 for m in range(CT)]
        bcy_cur = {}

        def stage_d(n):
            ns = slice(n * 512, (n + 1) * 512)
            ops = []
            for m in range(CT):
                def dproj(m=m):
                    ps = psum.tile([128, 512], F32, tag="spare",
                                   name=f"yp{m}{n}", bufs=2)
                    for t in range(CT):
                        nc.tensor.matmul(
                            out=ps[:, :],
                            lhsT=wo_sb[t][:, m * 128:(m + 1) * 128],
                            rhs=ao_sb[t][:, ns],
                            start=(t == 0), stop=(t == CT - 1))
                    ysl = ybig[:, m * L + n * 512: m * L + (n + 1) * 512]
                    nc.vector.tensor_scalar_add(ysl, ps[:, :], bo_sb[m])
                    nc.vector.tensor_mul(ysq_t[m][:, :], ysl, ysl)
                ops.append(dproj)

            dst_state = {}

            def dstat1():
                yr = psum.tile([128, 512], F32, tag="spare", bufs=2,
                               name=f"yr{n}")
                for m in range(CT):
                    nc.tensor.matmul(out=yr[0:1, :],
                                     lhsT=selxc_sb[:, 1:2],
                                     rhs=ysq_t[m][:, :],
                                     start=(m == 0), stop=(m == CT - 1))
                rowyb = pc.tile([1, 512], BF16, tag="rowyb", name=f"rwy{n}",
                                bufs=2)
                nc.vector.tensor_copy(rowyb[0:1, :], yr[0:1, :])
                syp = psum.tile([128, 512], F32, tag="spare", bufs=2,
                                name=f"syp{n}")
                for c in range(4):
                    nc.tensor.matmul(out=syp[:, c:c + 1],
                                     lhsT=rowyb[:, c * 128:(c + 1) * 128],
                                     rhs=identb_sb[0:1, 0:1],
                                     start=True, stop=True)
                dst_state["syp"] = syp
            ops.append(dstat1)

            def dstat2():
                syp = dst_state.pop("syp")
                ry = pc.tile([128, 4], F32, tag="ryq", name=f"ry{n}", bufs=2)
                dve_rsqrt(ry, syp, 4, C, f"ry{n}")
                ryb = pc.tile([128, 4], BF16, tag="rybq", name=f"ryb{n}",
                              bufs=2)
                nc.vector.tensor_copy(ryb[:, :], ry[:, :])
                ryrow_ps = psum.tile([128, 512], F32, tag="spare",
                                     name=f"ryr{n}", bufs=2)
                for c in range(4):
                    nc.tensor.matmul(out=ryrow_ps[0:1, c * 128:(c + 1) * 128],
                                     lhsT=ryb[:, c:c + 1],
                                     rhs=identb_sb[:, :],
                                     start=True, stop=True)
                ryrowb = pc.tile([1, 512], BF16, tag="ryrowb",
                                 name=f"ryrb{n}", bufs=2)
                nc.vector.tensor_copy(ryrowb[:, :], ryrow_ps[0:1, :])
                bcy = pc.tile([128, 512], BF16, tag="bcy", name=f"bcy{n}",
                              bufs=2)
                nc.gpsimd.partition_broadcast(bcy[:, :], ryrowb[0:1, :],
                                              channels=128)
                bcy_cur[0] = bcy
            ops.append(dstat2)

            for m in range(CT):
                def dfin(m=m):
                    ysl = ybig[:, m * L + n * 512: m * L + (n + 1) * 512]
                    tmp = pc.tile([128, 512], F32, tag="fintmp",
                                  name=f"ft{n}{m}", bufs=2)
                    nc.vector.scalar_tensor_tensor(
                        out=tmp[:, :], in0=ysl, scalar=g2_sb[m],
                        in1=bcy_cur[0][:, :], op0=OP.mult, op1=OP.mult)
                    nc.vector.tensor_add(ysl, tmp[:, :], x_sb[m][:, ns])
                    nc.sync.dma_start(
                        out=y_d[m * 128:(m + 1) * 128, ns], in_=ysl)
                ops.append(dfin)
            return ops

        # ---------------- attention ----------------------------------------
        pexp = top.enter_context(tc.tile_pool(name="exp", bufs=1))

        steps = [(n, p, j) for n in range(2) for p in range(4)
                 for j in range(JT)]

        sim_slots = {}
        ex_slots = {}

        def emit_sims(step):
            n, p, j = step
            ns = slice(n * 512, (n + 1) * 512)
            js = slice(j * 128, (j + 1) * 128)
            sl = psum.tile([128, 1024], F32, tag="sim", bufs=2,
                           name=f"sim{n}{p}{j}")
            nc.tensor.matmul(out=sl[:, 0:512],
                             lhsT=k_sb[p][0:64, js],
                             rhs=q_sb[p][0:64, ns],
                             start=True, stop=True)
            nc.tensor.matmul(out=sl[:, 512:1024],
                             lhsT=k_sb[p][64:128, js],
                             rhs=q_sb[p][64:128, ns],
                             start=True, stop=True)
            sim_slots[step] = sl

        def emit_exps(step):
            n, p, j = step
            ex = pexp.tile([128, 1024], BF16, tag="ex", bufs=4,
                           name=f"ex{n}{p}{j}")
            nc.scalar.activation(out=ex[:, :], in_=sim_slots.pop(step)[:, :],
                                 func=AF.Exp, bias=0.0,
                                 scale=rsq_xc[:, 2 * j:2 * j + 1])
            ex_slots[step] = ex
            if dbg and step == (0, 0, 0):
                exf = pc.tile([128, 1024], F32, tag="dbgexf")
                nc.vector.tensor_copy(exf[:, :], ex[:, :])
                nc.sync.dma_start(out=dbg_ex[:, :], in_=exf[:, :])

        ou_cur = {}

        def emit_pv(step):
            n, p, j = step
            if j == 0:
                ou_cur[0] = psum.tile([128, 512], F32, tag="ou", bufs=2,
                                      name=f"ou{n}{p}0")
                ou_cur[1] = psum.tile([128, 512], F32, tag="ou", bufs=2,
                                      name=f"ou{n}{p}1")
            ex = ex_slots.pop(step)
            for hi in range(2):
                h = 2 * p + hi  # global head
                nc.tensor.matmul(
                    out=ou_cur[hi][0:VW, :],
                    lhsT=vT_sb[j][:, h * VW:(h + 1) * VW],
                    rhs=ex[:, hi * 512:(hi + 1) * 512],
                    start=(j == 0), stop=(j == JT - 1))

        def emit_pair_norm1(step):
            # reciprocal of the denominator rows + partition broadcast.
            # gpsimd runs ONLY partition_broadcast (ext-isa lib) -- mixing
            # it with stock tensor ops thrashes the Q7 IRAM library.
            n, p, j = step
            ous, bcs = [], []
            for hi in range(2):
                ou = ou_cur[hi]
                rden = pc.tile([1, 512], F32, tag="rden",
                               name=f"rd{n}{p}{hi}", bufs=4)
                nc.vector.tensor_copy(rden[:, :], ou[HD:VW, :])
                rr = pc.tile([1, 512], F32, tag="rr", name=f"rr{n}{p}{hi}",
                             bufs=4)
                nc.vector.reciprocal_approx_fast(out=rr[:, :],
                                                 in_=rden[:, :])
                bc = pc.tile([HD, 512], F32, tag="bcd", name=f"bc{n}{p}{hi}",
                             bufs=4)
                nc.gpsimd.partition_broadcast(bc[:, :], rr[0:1, :],
                                              channels=HD)
                ous.append(ou)
                bcs.append(bc)
            return (ous, bcs)

        def emit_pair_norm2(step, ous, bcs):
            n, p, j = step
            ns = slice(n * 512, (n + 1) * 512)
            for hi in range(2):
                nc.vector.tensor_mul(
                    ao_sb[p][hi * HD:(hi + 1) * HD, ns],
                    ous[hi][0:HD, :], bcs[hi][:, :])

        # ---- emission with software pipelining ----
        d_ops = []
        pend2 = None
        emit_sims(steps[0])
        for si, step in enumerate(steps):
            n, p, j = step
            if si >= 36 and d_ops:
                d_ops.pop(0)()
            elif si >= 1 and filler:
                filler.pop(0)()
            emit_exps(step)
            if si + 1 < len(steps):
                emit_sims(steps[si + 1])
            if pend2 is not None:
                emit_pair_norm2(*pend2)
                pend2 = None
            emit_pv(step)
            if j == JT - 1:
                pend2 = (step,) + emit_pair_norm1(step)
                if (n, p) == (0, 3):
                    d_ops = stage_d(0)
        if pend2 is not None:
            emit_pair_norm2(*pend2)
        for op in d_ops:
            op()
        for op in stage_d(1):
            op()
        if dbg:
            aof = pc.tile([128, L], F32, tag="dbgaof")
            nc.vector.tensor_copy(aof[:, :], ao_sb[0][:, :])
            nc.sync.dma_start(out=dbg_ao[:, :], in_=aof[:, :])

    nc.compile()
    return nc


_NC_CACHE = {}


def _get_nc():
    if "nc" not in _NC_CACHE:
        _NC_CACHE["nc"] = build()
    return _NC_CACHE["nc"]


def kernel(x, context, Wq, Wkv, Wo, bo, g, g2):
    x = np.asarray(x, dtype=np.float32)
    context = np.asarray(context, dtype=np.float32)
    Wq = np.asarray(Wq, dtype=np.float32)
    Wkv = np.asarray(Wkv, dtype=np.float32)
    Wo = np.asarray(Wo, dtype=np.float32)
    bo = np.asarray(bo, dtype=np.float32)
    g = np.asarray(g, dtype=np.float32)
    g2 = np.asarray(g2, dtype=np.float32)

    bf = ml_dtypes.bfloat16
    scale = HD ** -0.5
    wq_h = np.ascontiguousarray((Wq * g[None, :] * scale).T).astype(bf)
    wk_h = np.ascontiguousarray((Wkv[:HID] * g[None, :]).T).astype(bf)
    wv_h = np.ascontiguousarray((Wkv[HID:] * g[None, :]).T).astype(bf)
    wo_h = np.ascontiguousarray(Wo.T).astype(bf)
    bog2T = np.ascontiguousarray(np.stack([bo, g2], axis=0))  # [2, C]
    identb = np.eye(128, dtype=np.float32).astype(bf)
    identr = np.eye(2, dtype=np.float32)
    # stats rows: ctx -> row 0 (rc at even cols after transpose),
    # x -> row 1 (rx at odd cols)
    selxc = np.zeros((128, 4), dtype=np.float32)
    selxc[:, 1] = 1.0   # x part (lhsT [:, 0:2]): col1 -> row 1
    selxc[:, 2] = 1.0   # ctx part (lhsT [:, 2:4]): col2 -> row 0
    selxc = selxc.astype(bf)

    nc = _get_nc()
    global _last_in_maps
    in_maps = []
    for i in range(NCORES):
        in_maps.append({
            "x": np.ascontiguousarray(x[i].reshape(C, L)).astype(bf),
            "ctxT": np.ascontiguousarray(context[i].T).astype(bf),
            "wq": wq_h, "wk": wk_h, "wv": wv_h, "wo": wo_h,
            "identb": identb, "identr": identr, "selxc": selxc,
            "bog2T": bog2T,
        })
    _last_in_maps = in_maps
    res = run_bass_kernel_spmd(nc, in_maps, list(range(NCORES)))
    out = np.stack([res.results[i]["y_out"].reshape(C, H, W)
                    for i in range(NCORES)])
    return out.astype(np.float32)


_last_in_maps = None


# revision 21
# speedup vs baseline: 1.0501x; 1.0363x over previous
"""Trainium2 Bass kernel for nn_CrossAttention (B=8, C=512, H=W=32, Lc=1024,
8 heads x 64 dim).

Sharding: data-parallel over batch B across the 8 NeuronCores (1 image/core,
no collectives). Feature-on-partitions layout; all matmuls contract over SBUF
partitions.

v2 restructure (from the 170us v1):
  - ACT (scalar engine) runs ONLY exp during the attention phase; the
    attention steady-state is ACT-paced at ~1.15us/step x 64 steps.
  - K(m0) projection runs as soon as the (ctx, wk) tiles land; K(m1..3)
    drain into attention PE slack as fillers.
  - All stats/aux matmuls are bf16 (squares, colsums, transposes).
  - Softmax normalization: DVE reciprocal of the denominator row,
    gpsimd partition_broadcast to [64, 512], one DVE mul from PSUM ->
    bf16 ao.  No PE broadcast matmuls, no PSUM->SBUF staging copies.
  - Per-pixel RMS rows (rx for q, ry for stage D) via tiny transpose-back
    matmuls + ones-bcast matmul (rx) / partition_broadcast (ry).
  - Squares + most evicts on DVE, residual adds + broadcasts on gpsimd.
  - fp32 x copy dropped: the residual uses the bf16 x (adds ~1e-3 rel).
"""

import numpy as np
import ml_dtypes
from contextlib import ExitStack

import concourse.bass as bass
from concourse import bacc
import concourse.mybir as mybir
import concourse.tile as tile
from concourse.bass_utils import run_bass_kernel_spmd

F32 = mybir.dt.float32
F32R = mybir.dt.float32r
BF16 = mybir.dt.bfloat16
I32 = mybir.dt.int32
AF = mybir.ActivationFunctionType
OP = mybir.AluOpType

B, C, H, W = 8, 512, 32, 32
L = H * W  # 1024 query pixels
LC = 1024  # context tokens
HEADS, HD = 8, 64
VW = HD + 1  # 65: v columns + ones column (emits softmax denominator)
HID = HEADS * HD  # 512
EPS = 1e-6
NCORES = 8
CT = C // 128  # 4 c-tiles
JT = LC // 128  # 8 j-tiles


def build(dbg=False):
    nc = bacc.Bacc("TRN2", target_bir_lowering=False, debug=False,
                   num_devices=NCORES)

    x_d = nc.dram_tensor("x", [C, L], BF16, kind="ExternalInput")
    ct_d = nc.dram_tensor("ctxT", [C, LC], BF16, kind="ExternalInput")
    wq_d = nc.dram_tensor("wq", [C, HID], BF16, kind="ExternalInput")
    wk_d = nc.dram_tensor("wk", [C, HID], BF16, kind="ExternalInput")
    wv_d = nc.dram_tensor("wv", [C, HID], BF16, kind="ExternalInput")
    wo_d = nc.dram_tensor("wo", [HID, C], BF16, kind="ExternalInput")
    identb_d = nc.dram_tensor("identb", [128, 128], BF16, kind="ExternalInput")
    identr_d = nc.dram_tensor("identr", [2, 2], F32R, kind="ExternalInput")
    selxc_d = nc.dram_tensor("selxc", [128, 4], BF16, kind="ExternalInput")
    bog2_d = nc.dram_tensor("bog2T", [2, C], F32R, kind="ExternalInput")
    y_d = nc.dram_tensor("y_out", [C, L], F32, kind="ExternalOutput")
    if dbg:
        dbg_rsq = nc.dram_tensor("dbg_rsq", [128, 16], F32, kind="ExternalOutput")
        dbg_k0 = nc.dram_tensor("dbg_k0", [128, LC], F32, kind="ExternalOutput")
        dbg_q0 = nc.dram_tensor("dbg_q0", [128, L], F32, kind="ExternalOutput")
        dbg_rr = nc.dram_tensor("dbg_rr", [2, 512], F32, kind="ExternalOutput")
        dbg_bc = nc.dram_tensor("dbg_bc", [HD, 512], F32, kind="ExternalOutput")
        dbg_ao = nc.dram_tensor("dbg_ao", [128, L], F32, kind="ExternalOutput")
        dbg_ex = nc.dram_tensor("dbg_ex", [128, 1024], F32, kind="ExternalOutput")

    with tile.TileContext(nc) as tc, ExitStack() as top:
        pc = top.enter_context(tc.tile_pool(name="main", bufs=1))
        psum = top.enter_context(tc.tile_pool(name="ps", bufs=1, space="PSUM"))

        # ---------------- input DMAs ----------------
        # sync ring: x + ctx interleaved (stats need x early, K needs ctx),
        # then the last wq tiles.  scalar ring: wk + the other ctx + first
        # wq (its queue then frees for exp).  gpsimd SWDGE: wk2/3, consts,
        # wv, wo.
        x_sb = [pc.tile([128, L], BF16, tag=f"x{t}", name=f"x{t}")
                for t in range(CT)]
        ct_sb = [pc.tile([128, LC], BF16, tag=f"ct{t}", name=f"ct{t}")
                 for t in range(CT)]
        wq_sb = [pc.tile([128, HID], BF16, tag=f"wq{t}", name=f"wq{t}")
                 for t in range(CT)]
        wk_sb = [pc.tile([128, HID], BF16, tag=f"wk{t}", name=f"wk{t}")
                 for t in range(CT)]
        wv_sb = [pc.tile([128, HID], BF16, tag=f"wv{t}", name=f"wv{t}")
                 for t in range(CT)]
        wo_sb = [pc.tile([128, C], BF16, tag=f"wo{t}", name=f"wo{t}")
                 for t in range(CT)]

        warm_b = pc.tile([128, 128], BF16, tag="warmb")
        nc.vector.memset(warm_b, 1.0)
        warm_ex = pc.tile([1, 8], BF16, tag="warmex")
        nc.scalar.activation(out=warm_ex[:, :], in_=warm_b[0:1, 0:8],
                             func=AF.Exp, bias=0.0, scale=0.0)

        def dma_in(eng, sb, dram, t):
            eng.dma_start(out=sb[t], in_=dram[t * 128:(t + 1) * 128, :])

        # sync: x tiles then wq2/3; scalar: wk0, ct0, wk1, ct1, wq0/1;
        # gpsimd SWDGE: consts, wk2/ct2, wk3/ct3, wv, wo, bog2.
        dma_in(nc.sync, x_sb, x_d, 0)
        dma_in(nc.sync, x_sb, x_d, 1)
        dma_in(nc.sync, x_sb, x_d, 2)
        dma_in(nc.sync, x_sb, x_d, 3)
        dma_in(nc.sync, wq_sb, wq_d, 2)
        dma_in(nc.sync, wq_sb, wq_d, 3)
        dma_in(nc.scalar, wk_sb, wk_d, 0)
        dma_in(nc.scalar, ct_sb, ct_d, 0)
        dma_in(nc.scalar, wk_sb, wk_d, 1)
        dma_in(nc.scalar, ct_sb, ct_d, 1)
        dma_in(nc.scalar, wq_sb, wq_d, 0)
        dma_in(nc.scalar, wq_sb, wq_d, 1)
        selxc_sb = pc.tile([128, 4], BF16, tag="selxc")
        nc.gpsimd.dma_start(out=selxc_sb, in_=selxc_d[:, :])
        identb_sb = pc.tile([128, 128], BF16, tag="identb")
        nc.gpsimd.dma_start(out=identb_sb, in_=identb_d[:, :])
        dma_in(nc.gpsimd, wk_sb, wk_d, 2)
        dma_in(nc.gpsimd, ct_sb, ct_d, 2)
        dma_in(nc.gpsimd, wk_sb, wk_d, 3)
        dma_in(nc.gpsimd, ct_sb, ct_d, 3)
        for t in range(CT):
            dma_in(nc.gpsimd, wv_sb, wv_d, t)
        for t in range(CT):
            dma_in(nc.gpsimd, wo_sb, wo_d, t)
        bog2_sb = pc.tile([2, C], F32R, tag="bog2")
        nc.gpsimd.dma_start(out=bog2_sb, in_=bog2_d[:, :])
        identr_sb = pc.tile([2, 2], F32R, tag="identr")
        nc.gpsimd.dma_start(out=identr_sb, in_=identr_d[:, :])
        # ext-isa library preload for partition_broadcast (the ~6us IRAM
        # load runs invisibly before this op; keep it off the DMA path)
        warm_bc = pc.tile([2, 8], BF16, tag="warmbc")
        nc.gpsimd.partition_broadcast(warm_bc[:, :], warm_b[0:1, 0:8],
                                      channels=2)

        # ---------------- warmup + exp table load --------------------------
        # warm_b is all-ones bf16: doubles as the ones operand for the PE
        # row-broadcast matmuls.
        warm_ps = psum.tile([128, 512], F32, tag="spare", name="warmps",
                            bufs=2)
        for i in range(24):
            nc.tensor.matmul(out=warm_ps[:, 0:128],
                             lhsT=warm_b[:, :], rhs=warm_b[:, :],
                             start=True, stop=True)

        # ---------------- squares (DVE, bf16 in/out) ------------------------
        sq_x, sq_c = [], []
        for t in range(CT):
            sx = pc.tile([128, L], BF16, tag=f"sqx{t}", name=f"sqx{t}")
            sc = pc.tile([128, LC], BF16, tag=f"sqc{t}", name=f"sqc{t}")
            sq_x.append(sx)
            sq_c.append(sc)
        for xt, ct in ((0, None), (None, 0), (1, None), (2, None),
                       (None, 2), (None, 1), (3, None), (None, 3)):
            if xt is not None:
                nc.vector.tensor_mul(sq_x[xt][:, :], x_sb[xt][:, :],
                                     x_sb[xt][:, :])
            else:
                nc.vector.tensor_mul(sq_c[ct][:, :], ct_sb[ct][:, :],
                                     ct_sb[ct][:, :])

        # ---------------- K(m0) projection (early) --------------------------
        k_sb = [pc.tile([128, LC], BF16, tag=f"k{m}", name=f"k{m}")
                for m in range(CT)]
        kp0 = []
        for h in range(2):
            kp = psum.tile([128, 512], F32, tag="ou", name=f"kp0{h}", bufs=2)
            kp0.append(kp)
        for ti, t in enumerate((0, 2, 1, 3)):
            for h in range(2):
                nc.tensor.matmul(out=kp0[h][:, :],
                                 lhsT=wk_sb[t][:, 0:128],
                                 rhs=ct_sb[t][:, h * 512:(h + 1) * 512],
                                 start=(ti == 0), stop=(ti == CT - 1))
        for h in range(2):
            nc.scalar.activation(out=k_sb[0][:, h * 512:(h + 1) * 512],
                                 in_=kp0[h][:, :], func=AF.Copy,
                                 bias=0.0, scale=1.0)

        # ---------------- stats colsums (bf16) ------------------------------
        # row0 = sum x^2 (per pixel), row1 = sum ctx^2 (per token); one
        # accumulation group per 512-col half, ctx parts first (they land
        # earlier than sq_x[3]).
        rows_ps = []
        for h in range(2):
            rp = psum.tile([128, 512], F32, tag="spare", name=f"rws{h}",
                           bufs=2)
            rows_ps.append(rp)
        for h in range(2):
            for t in range(CT):
                nc.tensor.matmul(out=rows_ps[h][0:2, :],
                                 lhsT=selxc_sb[:, 0:2],
                                 rhs=sq_x[t][:, h * 512:(h + 1) * 512],
                                 start=(t == 0), stop=False)
            for t in range(CT):
                nc.tensor.matmul(out=rows_ps[h][0:2, :],
                                 lhsT=selxc_sb[:, 2:4],
                                 rhs=sq_c[t][:, h * 512:(h + 1) * 512],
                                 start=False, stop=(t == CT - 1))
        rows2b = pc.tile([2, 1024], BF16, tag="rows2b")
        for h in range(2):
            nc.scalar.activation(out=rows2b[0:2, h * 512:(h + 1) * 512],
                                 in_=rows_ps[h][0:2, :], func=AF.Copy,
                                 bias=0.0, scale=1.0)

        # tiny transposes: [2, 128] chunks -> [128, 2] (ctx col, x col)
        ssq_ps = psum.tile([128, 512], F32, tag="spare", name="ssqps", bufs=2)
        for c in range(8):
            nc.tensor.matmul(out=ssq_ps[:, 2 * c:2 * c + 2],
                             lhsT=rows2b[:, c * 128:(c + 1) * 128],
                             rhs=identb_sb[0:2, 0:2],
                             start=True, stop=True)

        # Quake rsqrt on DVE: dst = (src/nfeat + eps)^-0.5, one Newton
        # pass (~0.2% max err; the downstream tolerance absorbs it).
        def dve_rsqrt(dst, src_ps, ncols, nfeat, scratch_tag):
            m = pc.tile([128, ncols], F32, tag=f"{scratch_tag}m")
            nc.vector.tensor_scalar(out=m[:, :], in0=src_ps[:, 0:ncols],
                                    scalar1=1.0 / nfeat, scalar2=EPS,
                                    op0=OP.mult, op1=OP.add)
            m2 = pc.tile([128, ncols], F32, tag=f"{scratch_tag}m2")
            nc.vector.tensor_scalar(out=m2[:, :], in0=src_ps[:, 0:ncols],
                                    scalar1=0.5 / nfeat, scalar2=0.5 * EPS,
                                    op0=OP.mult, op1=OP.add)
            i_f = pc.tile([128, ncols], F32, tag=f"{scratch_tag}if")
            nc.vector.tensor_copy(i_f[:, :], m[:, :].bitcast(I32))
            y0f = pc.tile([128, ncols], F32, tag=f"{scratch_tag}y0f")
            nc.vector.tensor_scalar(out=y0f[:, :], in0=i_f[:, :],
                                    scalar1=-0.5, scalar2=1.5974630e9,
                                    op0=OP.mult, op1=OP.add)
            y0 = pc.tile([128, ncols], I32, tag=f"{scratch_tag}y0")
            nc.vector.tensor_copy(y0[:, :], y0f[:, :])
            y0 = y0[:, :].bitcast(F32)
            t1 = pc.tile([128, ncols], F32, tag=f"{scratch_tag}t1")
            nc.vector.tensor_mul(t1[:, :], y0, y0)
            nc.vector.tensor_mul(t1[:, :], t1[:, :], m2[:, :])
            nc.vector.scalar_tensor_tensor(
                out=dst[:, :], in0=t1[:, :], scalar=1.5, in1=y0,
                op0=OP.subtract, op1=OP.mult)

        # rsq_xc cols: even = rc token chunks, odd = rx pixel chunks
        # (selxc col2 = ctx row first -> row0 = ctx, row1 = x after the
        # transpose: even col = ctx (rc), odd col = x (rx))
        rsq_xc = pc.tile([128, 16], F32, tag="rsqxc")
        dve_rsqrt(rsq_xc[:, 0:16], ssq_ps[:, 0:16], 16, C, "rs")
        rsqb = pc.tile([128, 16], BF16, tag="rsqb")
        nc.vector.tensor_copy(rsqb[:, :], rsq_xc[:, :])
        if dbg:
            nc.sync.dma_start(out=dbg_rsq[:, :], in_=rsq_xc[:, :])

        # rx broadcast: diag(rx-chunk) on DVE, then ones-bcast matmuls
        # (warm_b is all-ones); evict in two halves so q(n=0) starts early.
        bcx_ps = psum.tile([128, 1024], F32, tag="sim", name="bcxps", bufs=2)
        for c in range(8):
            dg = pc.tile([128, 128], BF16, tag="diag", name=f"dg{c}", bufs=4)
            nc.vector.tensor_scalar_mul(dg[:, :], identb_sb[:, :],
                                        rsq_xc[:, 2 * c + 1:2 * c + 2])
            nc.tensor.matmul(out=bcx_ps[:, c * 128:(c + 1) * 128],
                             lhsT=warm_b[:, :], rhs=dg[:, :],
                             start=True, stop=True)
        bcx_sb = pc.tile([128, 1024], BF16, tag="bcx")
        for h in range(2):
            nc.scalar.activation(out=bcx_sb[:, h * 512:(h + 1) * 512],
                                 in_=bcx_ps[:, h * 512:(h + 1) * 512],
                                 func=AF.Copy, bias=0.0, scale=1.0)

        # ---------------- projection machinery -----------------------------
        q_sb = [pc.tile([128, L], BF16, tag=f"q{m}", name=f"q{m}")
                for m in range(CT)]
        vT_sb = []
        for j in range(JT):
            vt = pc.tile([128, HEADS * VW], BF16, tag=f"vT{j}", name=f"vT{j}")
            vh = vt[:, :].rearrange("p (h c) -> p h c", h=HEADS)
            nc.vector.memset(vh[:, :, HD:VW], 1.0)
            vT_sb.append(vt)
        ao_sb = [pc.tile([128, L], BF16, tag=f"ao{m}", name=f"ao{m}")
                 for m in range(CT)]

        def proj_q(m, n):
            ns = slice(n * 512, (n + 1) * 512)
            ps = psum.tile([128, 512], F32, tag="spare", name=f"qp{m}{n}",
                           bufs=2)
            for t in range(CT):
                nc.tensor.matmul(out=ps[:, :],
                                 lhsT=wq_sb[t][:, m * 128:(m + 1) * 128],
                                 rhs=x_sb[t][:, ns],
                                 start=(t == 0), stop=(t == CT - 1))
            nc.vector.tensor_mul(q_sb[m][:, ns], ps[:, :], bcx_sb[:, ns])

        def proj_v(j):
            ps = psum.tile([128, HID], F32, tag="spare", name=f"vp{j}",
                           bufs=2)
            for t in range(CT):
                nc.tensor.matmul(out=ps[:, :],
                                 lhsT=ct_sb[t][:, j * 128:(j + 1) * 128],
                                 rhs=wv_sb[t][:, :],
                                 start=(t == 0), stop=(t == CT - 1))
            vh = vT_sb[j][:, :].rearrange("p (h c) -> p h c", h=HEADS)
            nc.vector.tensor_scalar_mul(
                vh[:, :, 0:HD],
                ps[:, :].rearrange("p (h c) -> p h c", h=HEADS),
                rsq_xc[:, 2 * j:2 * j + 1])

        def proj_k_half(m, h):
            ps = psum.tile([128, 512], F32, tag="spare", name=f"kp{m}{h}",
                           bufs=2)
            for t in range(CT):
                nc.tensor.matmul(out=ps[:, :],
                                 lhsT=wk_sb[t][:, m * 128:(m + 1) * 128],
                                 rhs=ct_sb[t][:, h * 512:(h + 1) * 512],
                                 start=(t == 0), stop=(t == CT - 1))
            nc.vector.tensor_copy(k_sb[m][:, h * 512:(h + 1) * 512],
                                  ps[:, :])

        def bog_transposes():
            bog_res = []
            for t in range(CT):
                bps = psum.tile([128, 512], F32, tag="spare", name=f"bog{t}",
                                bufs=2)
                nc.tensor.matmul(out=bps[:, 0:2],
                                 lhsT=bog2_sb[:, t * 128:(t + 1) * 128],
                                 rhs=identr_sb[0:2, 0:2],
                                 start=True, stop=True)
                bg = pc.tile([128, 2], F32, tag=f"bog2s{t}")
                nc.vector.tensor_copy(bg[:, :], bps[:, 0:2])
                bog_res.append(bg)
            for bg in bog_res:
                bo_sb.append(bg[:, 0:1])
                g2_sb.append(bg[:, 1:2])
        bo_sb, g2_sb = [], []

        # pre-attention projections (q n=0 and the first v tiles)
        proj_q(0, 0)
        proj_q(1, 0)
        proj_v(0)
        proj_v(1)
        proj_q(2, 0)
        proj_q(3, 0)

        # deferred work, drained into attention PE slack.  Order matters:
        # vT[j] must be emitted before the PV that reads it (PV(0,0,j) at
        # step j), k_sb[p] before pair p's sims (emitted one step early).
        filler = [
            lambda: proj_v(2),
            lambda: proj_v(3),
            lambda: proj_v(4),
            lambda: proj_v(5),
            lambda: proj_v(6),
            lambda: proj_v(7),
            lambda: proj_k_half(1, 0),
            lambda: proj_k_half(1, 1),
            lambda: proj_k_half(2, 0),
            bog_transposes,
            lambda: proj_k_half(2, 1),
            lambda: proj_k_half(3, 0),
            lambda: proj_k_half(3, 1),
            lambda: proj_q(0, 1),
            lambda: proj_q(1, 1),
            lambda: proj_q(2, 1),
            lambda: proj_q(3, 1),
        ]

        # ---------------- stage D (emitted later, per n) --------------------
        ybig = pc.tile([128, 4 * L], F32, tag="ybig")
        ysq_t = [pc.tile([128, 512], BF16, tag=f"ysq{m}", name=f"ysq{m}")
                 for m in range(CT)]
        bcy_cur = {}

        def stage_d(n, pre=None):
            ns = slice(n * 512, (n + 1) * 512)
            ops = []
            for m in range(CT):
                def dproj(m=m):
                    if pre is not None and m in pre:
                        ps = pre.pop(m)
                        nc.tensor.matmul(
                            out=ps[:, :],
                            lhsT=wo_sb[CT - 1][:, m * 128:(m + 1) * 128],
                            rhs=ao_sb[CT - 1][:, ns],
                            start=False, stop=True)
                    else:
                        ps = psum.tile([128, 512], F32, tag="spare",
                                       name=f"yp{m}{n}", bufs=2)
                        for t in range(CT):
                            nc.tensor.matmul(
                                out=ps[:, :],
                                lhsT=wo_sb[t][:, m * 128:(m + 1) * 128],
                                rhs=ao_sb[t][:, ns],
                                start=(t == 0), stop=(t == CT - 1))
                    ysl = ybig[:, m * L + n * 512: m * L + (n + 1) * 512]
                    nc.vector.tensor_scalar_add(ysl, ps[:, :], bo_sb[m])
                    nc.vector.tensor_mul(ysq_t[m][:, :], ysl, ysl)
                ops.append(dproj)

            dst_state = {}

            def dstat1():
                yr = psum.tile([128, 512], F32, tag="spare", bufs=2,
                               name=f"yr{n}")
                for m in range(CT):
                    nc.tensor.matmul(out=yr[0:1, :],
                                     lhsT=selxc_sb[:, 1:2],
                                     rhs=ysq_t[m][:, :],
                                     start=(m == 0), stop=(m == CT - 1))
                rowyb = pc.tile([1, 512], BF16, tag="rowyb", name=f"rwy{n}",
                                bufs=2)
                nc.vector.tensor_copy(rowyb[0:1, :], yr[0:1, :])
                syp = psum.tile([128, 512], F32, tag="spare", bufs=2,
                                name=f"syp{n}")
                for c in range(4):
                    nc.tensor.matmul(out=syp[:, c:c + 1],
                                     lhsT=rowyb[:, c * 128:(c + 1) * 128],
                                     rhs=identb_sb[0:1, 0:1],
                                     start=True, stop=True)
                dst_state["syp"] = syp
            ops.append(dstat1)

            def dstat2():
                syp = dst_state.pop("syp")
                ry = pc.tile([128, 4], F32, tag="ryq", name=f"ry{n}", bufs=2)
                dve_rsqrt(ry, syp, 4, C, f"ry{n}")
                ryb = pc.tile([128, 4], BF16, tag="rybq", name=f"ryb{n}",
                              bufs=2)
                nc.vector.tensor_copy(ryb[:, :], ry[:, :])
                ryrow_ps = psum.tile([128, 512], F32, tag="spare",
                                     name=f"ryr{n}", bufs=2)
                for c in range(4):
                    nc.tensor.matmul(out=ryrow_ps[0:1, c * 128:(c + 1) * 128],
                                     lhsT=ryb[:, c:c + 1],
                                     rhs=identb_sb[:, :],
                                     start=True, stop=True)
                ryrowb = pc.tile([1, 512], BF16, tag="ryrowb",
                                 name=f"ryrb{n}", bufs=2)
                nc.vector.tensor_copy(ryrowb[:, :], ryrow_ps[0:1, :])
                bcy = pc.tile([128, 512], BF16, tag="bcy", name=f"bcy{n}",
                              bufs=2)
                nc.gpsimd.partition_broadcast(bcy[:, :], ryrowb[0:1, :],
                                              channels=128)
                bcy_cur[0] = bcy
            ops.append(dstat2)

            for m in range(CT):
                def dfin(m=m):
                    ysl = ybig[:, m * L + n * 512: m * L + (n + 1) * 512]
                    tmp = pc.tile([128, 512], F32, tag="fintmp",
                                  name=f"ft{n}{m}", bufs=2)
                    nc.vector.scalar_tensor_tensor(
                        out=tmp[:, :], in0=ysl, scalar=g2_sb[m],
                        in1=bcy_cur[0][:, :], op0=OP.mult, op1=OP.mult)
                    nc.vector.tensor_add(ysl, tmp[:, :], x_sb[m][:, ns])
                    nc.sync.dma_start(
                        out=y_d[m * 128:(m + 1) * 128, ns], in_=ysl)
                ops.append(dfin)
            return ops

        # ---------------- attention ----------------------------------------
        pexp = top.enter_context(tc.tile_pool(name="exp", bufs=1))

        steps = [(n, p, j) for n in range(2) for p in range(4)
                 for j in range(JT)]

        sim_slots = {}
        ex_slots = {}

        def emit_sims(step):
            n, p, j = step
            ns = slice(n * 512, (n + 1) * 512)
            js = slice(j * 128, (j + 1) * 128)
            sl = psum.tile([128, 1024], F32, tag="sim", bufs=2,
                           name=f"sim{n}{p}{j}")
            nc.tensor.matmul(out=sl[:, 0:512],
                             lhsT=k_sb[p][0:64, js],
                             rhs=q_sb[p][0:64, ns],
                             start=True, stop=True)
            nc.tensor.matmul(out=sl[:, 512:1024],
                             lhsT=k_sb[p][64:128, js],
                             rhs=q_sb[p][64:128, ns],
                             start=True, stop=True)
            sim_slots[step] = sl

        def emit_exps(step):
            n, p, j = step
            ex = pexp.tile([128, 1024], BF16, tag="ex", bufs=4,
                           name=f"ex{n}{p}{j}")
            nc.scalar.activation(out=ex[:, :], in_=sim_slots.pop(step)[:, :],
                                 func=AF.Exp, bias=0.0,
                                 scale=rsq_xc[:, 2 * j:2 * j + 1])
            ex_slots[step] = ex
            if dbg and step == (0, 0, 0):
                exf = pc.tile([128, 1024], F32, tag="dbgexf")
                nc.vector.tensor_copy(exf[:, :], ex[:, :])
                nc.sync.dma_start(out=dbg_ex[:, :], in_=exf[:, :])

        ou_cur = {}

        def emit_pv(step):
            n, p, j = step
            if j == 0:
                ou_cur[0] = psum.tile([128, 512], F32, tag="ou", bufs=2,
                                      name=f"ou{n}{p}0")
                ou_cur[1] = psum.tile([128, 512], F32, tag="ou", bufs=2,
                                      name=f"ou{n}{p}1")
            ex = ex_slots.pop(step)
            for hi in range(2):
                h = 2 * p + hi  # global head
                nc.tensor.matmul(
                    out=ou_cur[hi][0:VW, :],
                    lhsT=vT_sb[j][:, h * VW:(h + 1) * VW],
                    rhs=ex[:, hi * 512:(hi + 1) * 512],
                    start=(j == 0), stop=(j == JT - 1))

        def emit_pair_norm1(step):
            # reciprocal of the denominator rows + partition broadcast.
            # gpsimd runs ONLY partition_broadcast (ext-isa lib) -- mixing
            # it with stock tensor ops thrashes the Q7 IRAM library.
            n, p, j = step
            ous, bcs = [], []
            for hi in range(2):
                ou = ou_cur[hi]
                rden = pc.tile([1, 512], F32, tag="rden",
                               name=f"rd{n}{p}{hi}", bufs=4)
                nc.vector.tensor_copy(rden[:, :], ou[HD:VW, :])
                rr = pc.tile([1, 512], F32, tag="rr", name=f"rr{n}{p}{hi}",
                             bufs=4)
                nc.vector.reciprocal_approx_fast(out=rr[:, :],
                                                 in_=rden[:, :])
                bc = pc.tile([HD, 512], F32, tag="bcd", name=f"bc{n}{p}{hi}",
                             bufs=4)
                nc.gpsimd.partition_broadcast(bc[:, :], rr[0:1, :],
                                              channels=HD)
                ous.append(ou)
                bcs.append(bc)
            return (ous, bcs)

        def emit_pair_norm2(step, ous, bcs):
            n, p, j = step
            ns = slice(n * 512, (n + 1) * 512)
            for hi in range(2):
                nc.vector.tensor_mul(
                    ao_sb[p][hi * HD:(hi + 1) * HD, ns],
                    ous[hi][0:HD, :], bcs[hi][:, :])

        # ---- emission with software pipelining ----
        # n=1 Wo partials: ao chunks 0..2 (pairs 0-2 of n=1) are final
        # once norm2(1,2) has been emitted (step 56); pre-start their
        # accumulations during the last pair's steps so only the t=3
        # matmul remains for the tail.
        d1_pre_ps = {}

        def dproj1_pre(m):
            ps = psum.tile([128, 512], F32, tag="spare",
                           name=f"y1pre{m}", bufs=2)
            for t in range(CT - 1):
                nc.tensor.matmul(out=ps[:, :],
                                 lhsT=wo_sb[t][:, m * 128:(m + 1) * 128],
                                 rhs=ao_sb[t][:, 512:1024],
                                 start=(t == 0), stop=False)
            d1_pre_ps[m] = ps

        d1_pre = [lambda: dproj1_pre(0), lambda: dproj1_pre(1)]
        d_ops = []
        pend2 = None
        emit_sims(steps[0])
        for si, step in enumerate(steps):
            n, p, j = step
            if si >= 36 and si % 2 == 0 and d_ops:
                d_ops.pop(0)()
            elif si >= 58 and d1_pre:
                d1_pre.pop(0)()
            elif si >= 1 and filler:
                filler.pop(0)()
            emit_exps(step)
            if si + 1 < len(steps):
                emit_sims(steps[si + 1])
            if pend2 is not None:
                emit_pair_norm2(*pend2)
                pend2 = None
            emit_pv(step)
            if j == JT - 1:
                pend2 = (step,) + emit_pair_norm1(step)
                if (n, p) == (0, 3):
                    d_ops = stage_d(0)
        if pend2 is not None:
            emit_pair_norm2(*pend2)
        for op in d_ops:
            op()
        for op in stage_d(1, pre=d1_pre_ps):
            op()
        if dbg:
            aof = pc.tile([128, L], F32, tag="dbgaof")
            nc.vector.tensor_copy(aof[:, :], ao_sb[0][:, :])
            nc.sync.dma_start(out=dbg_ao[:, :], in_=aof[:, :])

    nc.compile()
    return nc


_NC_CACHE = {}


def _get_nc():
    if "nc" not in _NC_CACHE:
        _NC_CACHE["nc"] = build()
    return _NC_CACHE["nc"]


def kernel(x, context, Wq, Wkv, Wo, bo, g, g2):
    x = np.asarray(x, dtype=np.float32)
    context = np.asarray(context, dtype=np.float32)
    Wq = np.asarray(Wq, dtype=np.float32)
    Wkv = np.asarray(Wkv, dtype=np.float32)
    Wo = np.asarray(Wo, dtype=np.float32)
    bo = np.asarray(bo, dtype=np.float32)
    g = np.asarray(g, dtype=np.float32)
    g2 = np.asarray(g2, dtype=np.float32)

    bf = ml_dtypes.bfloat16
    scale = HD ** -0.5
    wq_h = np.ascontiguousarray((Wq * g[None, :] * scale).T).astype(bf)
    wk_h = np.ascontiguousarray((Wkv[:HID] * g[None, :]).T).astype(bf)
    wv_h = np.ascontiguousarray((Wkv[HID:] * g[None, :]).T).astype(bf)
    wo_h = np.ascontiguousarray(Wo.T).astype(bf)
    bog2T = np.ascontiguousarray(np.stack([bo, g2], axis=0))  # [2, C]
    identb = np.eye(128, dtype=np.float32).astype(bf)
    identr = np.eye(2, dtype=np.float32)
    # stats rows: ctx -> row 0 (rc at even cols after transpose),
    # x -> row 1 (rx at odd cols)
    selxc = np.zeros((128, 4), dtype=np.float32)
    selxc[:, 1] = 1.0   # x part (lhsT [:, 0:2]): col1 -> row 1
    selxc[:, 2] = 1.0   # ctx part (lhsT [:, 2:4]): col2 -> row 0
    selxc = selxc.astype(bf)

    nc = _get_nc()
    global _last_in_maps
    in_maps = []
    for i in range(NCORES):
        in_maps.append({
            "x": np.ascontiguousarray(x[i].reshape(C, L)).astype(bf),
            "ctxT": np.ascontiguousarray(context[i].T).astype(bf),
            "wq": wq_h, "wk": wk_h, "wv": wv_h, "wo": wo_h,
            "identb": identb, "identr": identr, "selxc": selxc,
            "bog2T": bog2T,
        })
    _last_in_maps = in_maps
    res = run_bass_kernel_spmd(nc, in_maps, list(range(NCORES)))
    out = np.stack([res.results[i]["y_out"].reshape(C, H, W)
                    for i in range(NCORES)])
    return out.astype(np.float32)


_last_in_maps = None


# revision 22
# speedup vs baseline: 1.0518x; 1.0016x over previous
"""Trainium2 Bass kernel for nn_CrossAttention (B=8, C=512, H=W=32, Lc=1024,
8 heads x 64 dim).

Sharding: data-parallel over batch B across the 8 NeuronCores (1 image/core,
no collectives). Feature-on-partitions layout; all matmuls contract over SBUF
partitions.

v2 restructure (from the 170us v1):
  - ACT (scalar engine) runs ONLY exp during the attention phase; the
    attention steady-state is ACT-paced at ~1.15us/step x 64 steps.
  - K(m0) projection runs as soon as the (ctx, wk) tiles land; K(m1..3)
    drain into attention PE slack as fillers.
  - All stats/aux matmuls are bf16 (squares, colsums, transposes).
  - Softmax normalization: DVE reciprocal of the denominator row,
    gpsimd partition_broadcast to [64, 512], one DVE mul from PSUM ->
    bf16 ao.  No PE broadcast matmuls, no PSUM->SBUF staging copies.
  - Per-pixel RMS rows (rx for q, ry for stage D) via tiny transpose-back
    matmuls + ones-bcast matmul (rx) / partition_broadcast (ry).
  - Squares + most evicts on DVE, residual adds + broadcasts on gpsimd.
  - fp32 x copy dropped: the residual uses the bf16 x (adds ~1e-3 rel).
"""

import numpy as np
import ml_dtypes
from contextlib import ExitStack

import concourse.bass as bass
from concourse import bacc
import concourse.mybir as mybir
import concourse.tile as tile
from concourse.bass_utils import run_bass_kernel_spmd

F32 = mybir.dt.float32
F32R = mybir.dt.float32r
BF16 = mybir.dt.bfloat16
I32 = mybir.dt.int32
AF = mybir.ActivationFunctionType
OP = mybir.AluOpType

B, C, H, W = 8, 512, 32, 32
L = H * W  # 1024 query pixels
LC = 1024  # context tokens
HEADS, HD = 8, 64
VW = HD + 1  # 65: v columns + ones column (emits softmax denominator)
HID = HEADS * HD  # 512
EPS = 1e-6
NCORES = 8
CT = C // 128  # 4 c-tiles
JT = LC // 128  # 8 j-tiles


def build(dbg=False):
    nc = bacc.Bacc("TRN2", target_bir_lowering=False, debug=False,
                   num_devices=NCORES)

    x_d = nc.dram_tensor("x", [C, L], BF16, kind="ExternalInput")
    ct_d = nc.dram_tensor("ctxT", [C, LC], BF16, kind="ExternalInput")
    wq_d = nc.dram_tensor("wq", [C, HID], BF16, kind="ExternalInput")
    wk_d = nc.dram_tensor("wk", [C, HID], BF16, kind="ExternalInput")
    wv_d = nc.dram_tensor("wv", [C, HID], BF16, kind="ExternalInput")
    wo_d = nc.dram_tensor("wo", [HID, C], BF16, kind="ExternalInput")
    identb_d = nc.dram_tensor("identb", [128, 128], BF16, kind="ExternalInput")
    identr_d = nc.dram_tensor("identr", [2, 2], F32R, kind="ExternalInput")
    selxc_d = nc.dram_tensor("selxc", [128, 4], BF16, kind="ExternalInput")
    bog2_d = nc.dram_tensor("bog2T", [2, C], F32R, kind="ExternalInput")
    y_d = nc.dram_tensor("y_out", [C, L], F32, kind="ExternalOutput")
    if dbg:
        dbg_rsq = nc.dram_tensor("dbg_rsq", [128, 16], F32, kind="ExternalOutput")
        dbg_k0 = nc.dram_tensor("dbg_k0", [128, LC], F32, kind="ExternalOutput")
        dbg_q0 = nc.dram_tensor("dbg_q0", [128, L], F32, kind="ExternalOutput")
        dbg_rr = nc.dram_tensor("dbg_rr", [2, 512], F32, kind="ExternalOutput")
        dbg_bc = nc.dram_tensor("dbg_bc", [HD, 512], F32, kind="ExternalOutput")
        dbg_ao = nc.dram_tensor("dbg_ao", [128, L], F32, kind="ExternalOutput")
        dbg_ex = nc.dram_tensor("dbg_ex", [128, 1024], F32, kind="ExternalOutput")

    with tile.TileContext(nc) as tc, ExitStack() as top:
        pc = top.enter_context(tc.tile_pool(name="main", bufs=1))
        psum = top.enter_context(tc.tile_pool(name="ps", bufs=1, space="PSUM"))

        # ---------------- input DMAs ----------------
        # sync ring: x + ctx interleaved (stats need x early, K needs ctx),
        # then the last wq tiles.  scalar ring: wk + the other ctx + first
        # wq (its queue then frees for exp).  gpsimd SWDGE: wk2/3, consts,
        # wv, wo.
        x_sb = [pc.tile([128, L], BF16, tag=f"x{t}", name=f"x{t}")
                for t in range(CT)]
        ct_sb = [pc.tile([128, LC], BF16, tag=f"ct{t}", name=f"ct{t}")
                 for t in range(CT)]
        wq_sb = [pc.tile([128, HID], BF16, tag=f"wq{t}", name=f"wq{t}")
                 for t in range(CT)]
        wk_sb = [pc.tile([128, HID], BF16, tag=f"wk{t}", name=f"wk{t}")
                 for t in range(CT)]
        wv_sb = [pc.tile([128, HID], BF16, tag=f"wv{t}", name=f"wv{t}")
                 for t in range(CT)]
        wo_sb = [pc.tile([128, C], BF16, tag=f"wo{t}", name=f"wo{t}")
                 for t in range(CT)]

        warm_b = pc.tile([128, 128], BF16, tag="warmb")
        nc.vector.memset(warm_b, 1.0)
        warm_ex = pc.tile([1, 8], BF16, tag="warmex")
        nc.scalar.activation(out=warm_ex[:, :], in_=warm_b[0:1, 0:8],
                             func=AF.Exp, bias=0.0, scale=0.0)

        def dma_in(eng, sb, dram, t):
            eng.dma_start(out=sb[t], in_=dram[t * 128:(t + 1) * 128, :])

        # sync: x tiles then wq2/3; scalar: wk0, ct0, wk1, ct1, wq0/1;
        # gpsimd SWDGE: consts, wk2/ct2, wk3/ct3, wv, wo, bog2.
        dma_in(nc.sync, x_sb, x_d, 0)
        dma_in(nc.sync, x_sb, x_d, 1)
        dma_in(nc.sync, x_sb, x_d, 2)
        dma_in(nc.sync, x_sb, x_d, 3)
        dma_in(nc.sync, wq_sb, wq_d, 2)
        dma_in(nc.sync, wq_sb, wq_d, 3)
        dma_in(nc.scalar, wk_sb, wk_d, 0)
        dma_in(nc.scalar, ct_sb, ct_d, 0)
        dma_in(nc.scalar, wk_sb, wk_d, 1)
        dma_in(nc.scalar, ct_sb, ct_d, 1)
        dma_in(nc.scalar, wq_sb, wq_d, 0)
        dma_in(nc.scalar, wq_sb, wq_d, 1)
        selxc_sb = pc.tile([128, 4], BF16, tag="selxc")
        nc.gpsimd.dma_start(out=selxc_sb, in_=selxc_d[:, :])
        identb_sb = pc.tile([128, 128], BF16, tag="identb")
        nc.gpsimd.dma_start(out=identb_sb, in_=identb_d[:, :])
        dma_in(nc.gpsimd, wk_sb, wk_d, 2)
        dma_in(nc.gpsimd, ct_sb, ct_d, 2)
        dma_in(nc.gpsimd, wk_sb, wk_d, 3)
        dma_in(nc.gpsimd, ct_sb, ct_d, 3)
        for t in range(CT):
            dma_in(nc.gpsimd, wv_sb, wv_d, t)
        for t in range(CT):
            dma_in(nc.gpsimd, wo_sb, wo_d, t)
        bog2_sb = pc.tile([2, C], F32R, tag="bog2")
        nc.gpsimd.dma_start(out=bog2_sb, in_=bog2_d[:, :])
        identr_sb = pc.tile([2, 2], F32R, tag="identr")
        nc.gpsimd.dma_start(out=identr_sb, in_=identr_d[:, :])
        # ext-isa library preload for partition_broadcast (the ~6us IRAM
        # load runs invisibly before this op; keep it off the DMA path)
        warm_bc = pc.tile([2, 8], BF16, tag="warmbc")
        nc.gpsimd.partition_broadcast(warm_bc[:, :], warm_b[0:1, 0:8],
                                      channels=2)

        # ---------------- warmup + exp table load --------------------------
        # warm_b is all-ones bf16: doubles as the ones operand for the PE
        # row-broadcast matmuls.
        warm_ps = psum.tile([128, 512], F32, tag="spare", name="warmps",
                            bufs=2)
        for i in range(24):
            nc.tensor.matmul(out=warm_ps[:, 0:128],
                             lhsT=warm_b[:, :], rhs=warm_b[:, :],
                             start=True, stop=True)

        # ---------------- squares (DVE, bf16 in/out) ------------------------
        sq_x, sq_c = [], []
        for t in range(CT):
            sx = pc.tile([128, L], BF16, tag=f"sqx{t}", name=f"sqx{t}")
            sc = pc.tile([128, LC], BF16, tag=f"sqc{t}", name=f"sqc{t}")
            sq_x.append(sx)
            sq_c.append(sc)
        for xt, ct in ((0, None), (None, 0), (1, None), (2, None),
                       (None, 2), (None, 1), (3, None), (None, 3)):
            if xt is not None:
                nc.vector.tensor_mul(sq_x[xt][:, :], x_sb[xt][:, :],
                                     x_sb[xt][:, :])
            else:
                nc.vector.tensor_mul(sq_c[ct][:, :], ct_sb[ct][:, :],
                                     ct_sb[ct][:, :])

        # ---------------- K(m0) projection (early) --------------------------
        k_sb = [pc.tile([128, LC], BF16, tag=f"k{m}", name=f"k{m}")
                for m in range(CT)]
        kp0 = []
        for h in range(2):
            kp = psum.tile([128, 512], F32, tag="ou", name=f"kp0{h}", bufs=2)
            kp0.append(kp)
        for ti, t in enumerate((0, 2, 1, 3)):
            for h in range(2):
                nc.tensor.matmul(out=kp0[h][:, :],
                                 lhsT=wk_sb[t][:, 0:128],
                                 rhs=ct_sb[t][:, h * 512:(h + 1) * 512],
                                 start=(ti == 0), stop=(ti == CT - 1))
        for h in range(2):
            nc.scalar.activation(out=k_sb[0][:, h * 512:(h + 1) * 512],
                                 in_=kp0[h][:, :], func=AF.Copy,
                                 bias=0.0, scale=1.0)

        # ---------------- stats colsums (bf16) ------------------------------
        # row0 = sum x^2 (per pixel), row1 = sum ctx^2 (per token); one
        # accumulation group per 512-col half, ctx parts first (they land
        # earlier than sq_x[3]).
        rows_ps = []
        for h in range(2):
            rp = psum.tile([128, 512], F32, tag="spare", name=f"rws{h}",
                           bufs=2)
            rows_ps.append(rp)
        for h in range(2):
            for t in range(CT):
                nc.tensor.matmul(out=rows_ps[h][0:2, :],
                                 lhsT=selxc_sb[:, 0:2],
                                 rhs=sq_x[t][:, h * 512:(h + 1) * 512],
                                 start=(t == 0), stop=False)
            for t in range(CT):
                nc.tensor.matmul(out=rows_ps[h][0:2, :],
                                 lhsT=selxc_sb[:, 2:4],
                                 rhs=sq_c[t][:, h * 512:(h + 1) * 512],
                                 start=False, stop=(t == CT - 1))
        rows2b = pc.tile([2, 1024], BF16, tag="rows2b")
        for h in range(2):
            nc.scalar.activation(out=rows2b[0:2, h * 512:(h + 1) * 512],
                                 in_=rows_ps[h][0:2, :], func=AF.Copy,
                                 bias=0.0, scale=1.0)

        # tiny transposes: [2, 128] chunks -> [128, 2] (ctx col, x col)
        ssq_ps = psum.tile([128, 512], F32, tag="spare", name="ssqps", bufs=2)
        for c in range(8):
            nc.tensor.matmul(out=ssq_ps[:, 2 * c:2 * c + 2],
                             lhsT=rows2b[:, c * 128:(c + 1) * 128],
                             rhs=identb_sb[0:2, 0:2],
                             start=True, stop=True)

        # Quake rsqrt on DVE: dst = (src/nfeat + eps)^-0.5, one Newton
        # pass (~0.2% max err; the downstream tolerance absorbs it).
        def dve_rsqrt(dst, src_ps, ncols, nfeat, scratch_tag):
            m = pc.tile([128, ncols], F32, tag=f"{scratch_tag}m")
            nc.vector.tensor_scalar(out=m[:, :], in0=src_ps[:, 0:ncols],
                                    scalar1=1.0 / nfeat, scalar2=EPS,
                                    op0=OP.mult, op1=OP.add)
            m2 = pc.tile([128, ncols], F32, tag=f"{scratch_tag}m2")
            nc.vector.tensor_scalar(out=m2[:, :], in0=src_ps[:, 0:ncols],
                                    scalar1=0.5 / nfeat, scalar2=0.5 * EPS,
                                    op0=OP.mult, op1=OP.add)
            i_f = pc.tile([128, ncols], F32, tag=f"{scratch_tag}if")
            nc.vector.tensor_copy(i_f[:, :], m[:, :].bitcast(I32))
            y0f = pc.tile([128, ncols], F32, tag=f"{scratch_tag}y0f")
            nc.vector.tensor_scalar(out=y0f[:, :], in0=i_f[:, :],
                                    scalar1=-0.5, scalar2=1.5974630e9,
                                    op0=OP.mult, op1=OP.add)
            y0 = pc.tile([128, ncols], I32, tag=f"{scratch_tag}y0")
            nc.vector.tensor_copy(y0[:, :], y0f[:, :])
            y0 = y0[:, :].bitcast(F32)
            t1 = pc.tile([128, ncols], F32, tag=f"{scratch_tag}t1")
            nc.vector.tensor_mul(t1[:, :], y0, y0)
            nc.vector.tensor_mul(t1[:, :], t1[:, :], m2[:, :])
            nc.vector.scalar_tensor_tensor(
                out=dst[:, :], in0=t1[:, :], scalar=1.5, in1=y0,
                op0=OP.subtract, op1=OP.mult)

        # rsq_xc cols: even = rc token chunks, odd = rx pixel chunks
        # (selxc col2 = ctx row first -> row0 = ctx, row1 = x after the
        # transpose: even col = ctx (rc), odd col = x (rx))
        rsq_xc = pc.tile([128, 16], F32, tag="rsqxc")
        dve_rsqrt(rsq_xc[:, 0:16], ssq_ps[:, 0:16], 16, C, "rs")
        rsqb = pc.tile([128, 16], BF16, tag="rsqb")
        nc.vector.tensor_copy(rsqb[:, :], rsq_xc[:, :])
        if dbg:
            nc.sync.dma_start(out=dbg_rsq[:, :], in_=rsq_xc[:, :])

        # rx broadcast: diag(rx-chunk) on DVE, then ones-bcast matmuls
        # (warm_b is all-ones); evict in two halves so q(n=0) starts early.
        bcx_ps = psum.tile([128, 1024], F32, tag="sim", name="bcxps", bufs=2)
        for c in range(8):
            dg = pc.tile([128, 128], BF16, tag="diag", name=f"dg{c}", bufs=4)
            nc.vector.tensor_scalar_mul(dg[:, :], identb_sb[:, :],
                                        rsq_xc[:, 2 * c + 1:2 * c + 2])
            nc.tensor.matmul(out=bcx_ps[:, c * 128:(c + 1) * 128],
                             lhsT=warm_b[:, :], rhs=dg[:, :],
                             start=True, stop=True)
        bcx_sb = pc.tile([128, 1024], BF16, tag="bcx")
        for h in range(2):
            nc.scalar.activation(out=bcx_sb[:, h * 512:(h + 1) * 512],
                                 in_=bcx_ps[:, h * 512:(h + 1) * 512],
                                 func=AF.Copy, bias=0.0, scale=1.0)

        # ---------------- projection machinery -----------------------------
        q_sb = [pc.tile([128, L], BF16, tag=f"q{m}", name=f"q{m}")
                for m in range(CT)]
        vT_sb = []
        for j in range(JT):
            vt = pc.tile([128, HEADS * VW], BF16, tag=f"vT{j}", name=f"vT{j}")
            vh = vt[:, :].rearrange("p (h c) -> p h c", h=HEADS)
            nc.vector.memset(vh[:, :, HD:VW], 1.0)
            vT_sb.append(vt)
        ao_sb = [pc.tile([128, L], BF16, tag=f"ao{m}", name=f"ao{m}")
                 for m in range(CT)]

        def proj_q(m, n):
            ns = slice(n * 512, (n + 1) * 512)
            ps = psum.tile([128, 512], F32, tag="spare", name=f"qp{m}{n}",
                           bufs=2)
            for t in range(CT):
                nc.tensor.matmul(out=ps[:, :],
                                 lhsT=wq_sb[t][:, m * 128:(m + 1) * 128],
                                 rhs=x_sb[t][:, ns],
                                 start=(t == 0), stop=(t == CT - 1))
            nc.vector.tensor_mul(q_sb[m][:, ns], ps[:, :], bcx_sb[:, ns])

        def proj_v(j):
            ps = psum.tile([128, HID], F32, tag="spare", name=f"vp{j}",
                           bufs=2)
            for t in range(CT):
                nc.tensor.matmul(out=ps[:, :],
                                 lhsT=ct_sb[t][:, j * 128:(j + 1) * 128],
                                 rhs=wv_sb[t][:, :],
                                 start=(t == 0), stop=(t == CT - 1))
            vh = vT_sb[j][:, :].rearrange("p (h c) -> p h c", h=HEADS)
            nc.vector.tensor_scalar_mul(
                vh[:, :, 0:HD],
                ps[:, :].rearrange("p (h c) -> p h c", h=HEADS),
                rsq_xc[:, 2 * j:2 * j + 1])

        def proj_k_half(m, h):
            ps = psum.tile([128, 512], F32, tag="spare", name=f"kp{m}{h}",
                           bufs=2)
            for t in range(CT):
                nc.tensor.matmul(out=ps[:, :],
                                 lhsT=wk_sb[t][:, m * 128:(m + 1) * 128],
                                 rhs=ct_sb[t][:, h * 512:(h + 1) * 512],
                                 start=(t == 0), stop=(t == CT - 1))
            nc.vector.tensor_copy(k_sb[m][:, h * 512:(h + 1) * 512],
                                  ps[:, :])

        def bog_transposes():
            bog_res = []
            for t in range(CT):
                bps = psum.tile([128, 512], F32, tag="spare", name=f"bog{t}",
                                bufs=2)
                nc.tensor.matmul(out=bps[:, 0:2],
                                 lhsT=bog2_sb[:, t * 128:(t + 1) * 128],
                                 rhs=identr_sb[0:2, 0:2],
                                 start=True, stop=True)
                bg = pc.tile([128, 2], F32, tag=f"bog2s{t}")
                nc.vector.tensor_copy(bg[:, :], bps[:, 0:2])
                bog_res.append(bg)
            for bg in bog_res:
                bo_sb.append(bg[:, 0:1])
                g2_sb.append(bg[:, 1:2])
        bo_sb, g2_sb = [], []

        # pre-attention projections (q n=0 and the first v tiles)
        proj_q(0, 0)
        proj_q(1, 0)
        proj_v(0)
        proj_v(1)
        proj_q(2, 0)
        proj_q(3, 0)

        # deferred work, drained into attention PE slack.  Order matters:
        # vT[j] must be emitted before the PV that reads it (PV(0,0,j) at
        # step j), k_sb[p] before pair p's sims (emitted one step early).
        filler = [
            lambda: proj_v(2),
            lambda: proj_v(3),
            lambda: proj_v(4),
            lambda: proj_v(5),
            lambda: proj_v(6),
            lambda: proj_v(7),
            lambda: proj_k_half(1, 0),
            lambda: proj_k_half(1, 1),
            lambda: proj_k_half(2, 0),
            bog_transposes,
            lambda: proj_k_half(2, 1),
            lambda: proj_k_half(3, 0),
            lambda: proj_k_half(3, 1),
            lambda: proj_q(0, 1),
            lambda: proj_q(1, 1),
            lambda: proj_q(2, 1),
            lambda: proj_q(3, 1),
        ]

        # ---------------- stage D (emitted later, per n) --------------------
        ybig = pc.tile([128, 4 * L], F32, tag="ybig")
        ysq_t = [pc.tile([128, 512], BF16, tag=f"ysq{m}", name=f"ysq{m}")
                 for m in range(CT)]
        bcy_cur = {}

        def stage_d(n, pre=None):
            ns = slice(n * 512, (n + 1) * 512)
            ops = []
            for m in range(CT):
                def dproj(m=m):
                    if pre is not None and m in pre:
                        ps = pre.pop(m)
                        nc.tensor.matmul(
                            out=ps[:, :],
                            lhsT=wo_sb[CT - 1][:, m * 128:(m + 1) * 128],
                            rhs=ao_sb[CT - 1][:, ns],
                            start=False, stop=True)
                    else:
                        ps = psum.tile([128, 512], F32, tag="spare",
                                       name=f"yp{m}{n}", bufs=2)
                        for t in range(CT):
                            nc.tensor.matmul(
                                out=ps[:, :],
                                lhsT=wo_sb[t][:, m * 128:(m + 1) * 128],
                                rhs=ao_sb[t][:, ns],
                                start=(t == 0), stop=(t == CT - 1))
                    ysl = ybig[:, m * L + n * 512: m * L + (n + 1) * 512]
                    nc.vector.tensor_scalar_add(ysl, ps[:, :], bo_sb[m])
                    nc.vector.tensor_mul(ysq_t[m][:, :], ysl, ysl)
                ops.append(dproj)

            dst_state = {}

            def dstat1():
                yr = psum.tile([128, 512], F32, tag="spare", bufs=2,
                               name=f"yr{n}")
                for m in range(CT):
                    nc.tensor.matmul(out=yr[0:1, :],
                                     lhsT=selxc_sb[:, 1:2],
                                     rhs=ysq_t[m][:, :],
                                     start=(m == 0), stop=(m == CT - 1))
                rowyb = pc.tile([1, 512], BF16, tag="rowyb", name=f"rwy{n}",
                                bufs=2)
                nc.vector.tensor_copy(rowyb[0:1, :], yr[0:1, :])
                syp = psum.tile([128, 512], F32, tag="spare", bufs=2,
                                name=f"syp{n}")
                for c in range(4):
                    nc.tensor.matmul(out=syp[:, c:c + 1],
                                     lhsT=rowyb[:, c * 128:(c + 1) * 128],
                                     rhs=identb_sb[0:1, 0:1],
                                     start=True, stop=True)
                dst_state["syp"] = syp
            ops.append(dstat1)

            def dstat2():
                syp = dst_state.pop("syp")
                ry = pc.tile([128, 4], F32, tag="ryq", name=f"ry{n}", bufs=2)
                dve_rsqrt(ry, syp, 4, C, f"ry{n}")
                ryb = pc.tile([128, 4], BF16, tag="rybq", name=f"ryb{n}",
                              bufs=2)
                nc.vector.tensor_copy(ryb[:, :], ry[:, :])
                ryrow_ps = psum.tile([128, 512], F32, tag="spare",
                                     name=f"ryr{n}", bufs=2)
                for c in range(4):
                    nc.tensor.matmul(out=ryrow_ps[0:1, c * 128:(c + 1) * 128],
                                     lhsT=ryb[:, c:c + 1],
                                     rhs=identb_sb[:, :],
                                     start=True, stop=True)
                ryrowb = pc.tile([1, 512], BF16, tag="ryrowb",
                                 name=f"ryrb{n}", bufs=2)
                nc.vector.tensor_copy(ryrowb[:, :], ryrow_ps[0:1, :])
                bcy = pc.tile([128, 512], BF16, tag="bcy", name=f"bcy{n}",
                              bufs=2)
                nc.gpsimd.partition_broadcast(bcy[:, :], ryrowb[0:1, :],
                                              channels=128)
                bcy_cur[0] = bcy
            ops.append(dstat2)

            for m in range(CT):
                def dfin(m=m):
                    ysl = ybig[:, m * L + n * 512: m * L + (n + 1) * 512]
                    tmp = pc.tile([128, 512], F32, tag="fintmp",
                                  name=f"ft{n}{m}", bufs=2)
                    nc.vector.scalar_tensor_tensor(
                        out=tmp[:, :], in0=ysl, scalar=g2_sb[m],
                        in1=bcy_cur[0][:, :], op0=OP.mult, op1=OP.mult)
                    nc.vector.tensor_add(ysl, tmp[:, :], x_sb[m][:, ns])
                    nc.sync.dma_start(
                        out=y_d[m * 128:(m + 1) * 128, ns], in_=ysl)
                ops.append(dfin)
            return ops

        # ---------------- attention ----------------------------------------
        pexp = top.enter_context(tc.tile_pool(name="exp", bufs=1))

        steps = [(n, p, j) for n in range(2) for p in range(4)
                 for j in range(JT)]

        sim_slots = {}
        ex_slots = {}

        def emit_sims(step):
            n, p, j = step
            ns = slice(n * 512, (n + 1) * 512)
            js = slice(j * 128, (j + 1) * 128)
            sl = psum.tile([128, 1024], F32, tag="sim", bufs=2,
                           name=f"sim{n}{p}{j}")
            nc.tensor.matmul(out=sl[:, 0:512],
                             lhsT=k_sb[p][0:64, js],
                             rhs=q_sb[p][0:64, ns],
                             start=True, stop=True)
            nc.tensor.matmul(out=sl[:, 512:1024],
                             lhsT=k_sb[p][64:128, js],
                             rhs=q_sb[p][64:128, ns],
                             start=True, stop=True)
            sim_slots[step] = sl

        def emit_exps(step):
            n, p, j = step
            ex = pexp.tile([128, 1024], BF16, tag="ex", bufs=4,
                           name=f"ex{n}{p}{j}")
            nc.scalar.activation(out=ex[:, :], in_=sim_slots.pop(step)[:, :],
                                 func=AF.Exp, bias=0.0,
                                 scale=rsq_xc[:, 2 * j:2 * j + 1])
            ex_slots[step] = ex
            if dbg and step == (0, 0, 0):
                exf = pc.tile([128, 1024], F32, tag="dbgexf")
                nc.vector.tensor_copy(exf[:, :], ex[:, :])
                nc.sync.dma_start(out=dbg_ex[:, :], in_=exf[:, :])

        ou_cur = {}

        def emit_pv(step):
            n, p, j = step
            if j == 0:
                ou_cur[0] = psum.tile([128, 512], F32, tag="ou", bufs=2,
                                      name=f"ou{n}{p}0")
                ou_cur[1] = psum.tile([128, 512], F32, tag="ou", bufs=2,
                                      name=f"ou{n}{p}1")
            ex = ex_slots.pop(step)
            for hi in range(2):
                h = 2 * p + hi  # global head
                nc.tensor.matmul(
                    out=ou_cur[hi][0:VW, :],
                    lhsT=vT_sb[j][:, h * VW:(h + 1) * VW],
                    rhs=ex[:, hi * 512:(hi + 1) * 512],
                    start=(j == 0), stop=(j == JT - 1))

        def emit_pair_norm1(step):
            # reciprocal of the denominator rows + partition broadcast.
            # gpsimd runs ONLY partition_broadcast (ext-isa lib) -- mixing
            # it with stock tensor ops thrashes the Q7 IRAM library.
            n, p, j = step
            ous, bcs = [], []
            for hi in range(2):
                ou = ou_cur[hi]
                rden = pc.tile([1, 512], F32, tag="rden",
                               name=f"rd{n}{p}{hi}", bufs=4)
                nc.vector.tensor_copy(rden[:, :], ou[HD:VW, :])
                rr = pc.tile([1, 512], F32, tag="rr", name=f"rr{n}{p}{hi}",
                             bufs=4)
                nc.vector.reciprocal_approx_fast(out=rr[:, :],
                                                 in_=rden[:, :])
                bc = pc.tile([HD, 512], F32, tag="bcd", name=f"bc{n}{p}{hi}",
                             bufs=4)
                nc.gpsimd.partition_broadcast(bc[:, :], rr[0:1, :],
                                              channels=HD)
                ous.append(ou)
                bcs.append(bc)
            return (ous, bcs)

        def emit_pair_norm2(step, ous, bcs):
            n, p, j = step
            ns = slice(n * 512, (n + 1) * 512)
            for hi in range(2):
                nc.vector.tensor_mul(
                    ao_sb[p][hi * HD:(hi + 1) * HD, ns],
                    ous[hi][0:HD, :], bcs[hi][:, :])

        # ---- emission with software pipelining ----
        # n=1 Wo partials: ao chunks 0..2 (pairs 0-2 of n=1) are final
        # once norm2(1,2) has been emitted (step 56); pre-start their
        # accumulations during the last pair's steps so only the t=3
        # matmul remains for the tail.
        d1_pre_ps = {}

        def dproj1_pre(m):
            ps = psum.tile([128, 512], F32, tag="spare",
                           name=f"y1pre{m}", bufs=2)
            for t in range(CT - 1):
                nc.tensor.matmul(out=ps[:, :],
                                 lhsT=wo_sb[t][:, m * 128:(m + 1) * 128],
                                 rhs=ao_sb[t][:, 512:1024],
                                 start=(t == 0), stop=False)
            d1_pre_ps[m] = ps

        d1_pre = [lambda: dproj1_pre(0), lambda: dproj1_pre(1)]
        d_ops = []
        pend2 = None
        emit_sims(steps[0])
        for si, step in enumerate(steps):
            n, p, j = step
            # v2..v7 + k10 must precede this step's sims/PVs; everything
            # else drains at the bottom so the exp stream isn't delayed.
            if 1 <= si <= 7 and filler:
                filler.pop(0)()
            emit_exps(step)
            if si + 1 < len(steps):
                emit_sims(steps[si + 1])
            if pend2 is not None:
                emit_pair_norm2(*pend2)
                pend2 = None
            emit_pv(step)
            if j == JT - 1:
                pend2 = (step,) + emit_pair_norm1(step)
                if (n, p) == (0, 3):
                    d_ops = stage_d(0)
            if si >= 36 and si % 2 == 0 and d_ops:
                d_ops.pop(0)()
            elif si >= 58 and d1_pre:
                d1_pre.pop(0)()
            elif si >= 8 and filler:
                filler.pop(0)()
        if pend2 is not None:
            emit_pair_norm2(*pend2)
        for op in d_ops:
            op()
        for op in stage_d(1, pre=d1_pre_ps):
            op()
        if dbg:
            aof = pc.tile([128, L], F32, tag="dbgaof")
            nc.vector.tensor_copy(aof[:, :], ao_sb[0][:, :])
            nc.sync.dma_start(out=dbg_ao[:, :], in_=aof[:, :])

    nc.compile()
    return nc


_NC_CACHE = {}


def _get_nc():
    if "nc" not in _NC_CACHE:
        _NC_CACHE["nc"] = build()
    return _NC_CACHE["nc"]


def kernel(x, context, Wq, Wkv, Wo, bo, g, g2):
    x = np.asarray(x, dtype=np.float32)
    context = np.asarray(context, dtype=np.float32)
    Wq = np.asarray(Wq, dtype=np.float32)
    Wkv = np.asarray(Wkv, dtype=np.float32)
    Wo = np.asarray(Wo, dtype=np.float32)
    bo = np.asarray(bo, dtype=np.float32)
    g = np.asarray(g, dtype=np.float32)
    g2 = np.asarray(g2, dtype=np.float32)

    bf = ml_dtypes.bfloat16
    scale = HD ** -0.5
    wq_h = np.ascontiguousarray((Wq * g[None, :] * scale).T).astype(bf)
    wk_h = np.ascontiguousarray((Wkv[:HID] * g[None, :]).T).astype(bf)
    wv_h = np.ascontiguousarray((Wkv[HID:] * g[None, :]).T).astype(bf)
    wo_h = np.ascontiguousarray(Wo.T).astype(bf)
    bog2T = np.ascontiguousarray(np.stack([bo, g2], axis=0))  # [2, C]
    identb = np.eye(128, dtype=np.float32).astype(bf)
    identr = np.eye(2, dtype=np.float32)
    # stats rows: ctx -> row 0 (rc at even cols after transpose),
    # x -> row 1 (rx at odd cols)
    selxc = np.zeros((128, 4), dtype=np.float32)
    selxc[:, 1] = 1.0   # x part (lhsT [:, 0:2]): col1 -> row 1
    selxc[:, 2] = 1.0   # ctx part (lhsT [:, 2:4]): col2 -> row 0
    selxc = selxc.astype(bf)

    nc = _get_nc()
    global _last_in_maps
    in_maps = []
    for i in range(NCORES):
        in_maps.append({
            "x": np.ascontiguousarray(x[i].reshape(C, L)).astype(bf),
            "ctxT": np.ascontiguousarray(context[i].T).astype(bf),
            "wq": wq_h, "wk": wk_h, "wv": wv_h, "wo": wo_h,
            "identb": identb, "identr": identr, "selxc": selxc,
            "bog2T": bog2T,
        })
    _last_in_maps = in_maps
    res = run_bass_kernel_spmd(nc, in_maps, list(range(NCORES)))
    out = np.stack([res.results[i]["y_out"].reshape(C, H, W)
                    for i in range(NCORES)])
    return out.astype(np.float32)


_last_in_maps = None
